# revision 13
# baseline (speedup 1.0000x reference)
"""2-layer GCN (improved=True) + linear head + softmax on 8 Trainium2 cores.

Strategy (dest-node partitioning, v4: class-phased gather pipeline,
4-tile gather groups):
- Nodes assigned to 8 cores x 49 tiles x 128 slots via balanced bin-packing.
  Self-loops are never gathered: every XW output tile stays resident in SBUF
  (t_all) and the self term is one extra matmul against a per-tile diagonal
  (2*dis_dst) kept in the B matrix.
- Normalization fully folded into data: table rows are dis_src * XW (scaled
  at the PSUM->SBUF copy), the one-hot scatter matrix B holds fp8(dis_dst),
  so aggregation needs no post-scaling at all.
- Table is region-major: region 0 = tiles 0..15 (rows 0:16384), region 1 =
  tiles 16..31 (16384:32768), region 2 = tiles 32..48 (32768:50176).  Each
  layer runs THREE AllGathers (one per region).  Triggers are interleaved
  into the gpsimd gather-issue order mid-stream so they dispatch promptly
  (the gpsimd engine is in-order) while their semaphore deps keep them
  exact.
- Per dest tile, edges go to 3 gather classes with fixed chunk budgets:
  lo-a (5 chunks, sources < 16384, dep AG r0), lo-b (5 chunks, sources <
  32768, dep AG r1), hi (7 chunks, sources >= 17408, dep AG r2).  Gathers
  cover FOUR tiles per instruction (amortizing per-instruction overhead)
  on a strict 4-queue SWDGE rotation, issued class-major.
- Compute is phased per layer: all lo-a chunk matmul groups first
  (recycling gather buffers while AG r1/r2 are in flight), then all lo-b
  groups, then per-tile hi+self groups plus the nonlinearity and the
  next-layer XW.  Each class group drains PSUM into an SBUF f32
  accumulator (acc_all), so only a few PSUM banks are ever live.
- B matrices: even tiles resident in SBUF; odd tiles streamed per class
  with a small lookahead on the scalar queue.
- Head: logits = H2 @ Wlin + blin, softmax over 8 classes.

kernel() is self-contained: host-side numpy does all graph preprocessing;
the device program is identical on all 8 cores, only data differs.
"""
import sys

sys.path.insert(0, "/opt/trn_rl_repo")

import numpy as np
import ml_dtypes

import concourse.bass as bass
import concourse.bacc as bacc
import concourse.mybir as mybir
import concourse.tile as tile
from concourse.tile_rust import add_dep_helper
from concourse.bass_utils import run_bass_kernel_spmd
from concourse.library_config import mlp

# problem constants
N = 50000
E = 800000
FIN = 512
D = 128
NCLS = 8
NCORES = 8

# sharding constants
P = 128
TILES = 49
NLOC = TILES * P            # 6272 slots per core
VTOT = NCORES * NLOC        # 50176 table rows

GRP = 4                     # tiles per gather group
NGRP = (TILES + GRP - 1) // GRP     # 13 (last group has 1 tile)

# region-major table: region r holds tiles RT[r][0]..RT[r][1] of every core
RT = [(0, 16), (16, 32), (32, TILES)]
RROWS = [(b - a) * P for a, b in RT]           # rows per core per region
RBASE = [0, 16384, 32768]                      # global row base per region
HI_BASE = 17408                                # hi gathers read table[HI_BASE:]

# gather classes: chunks per tile
CH_A = 5                    # sources < 16384            (needs AG r0)
CH_B = 5                    # sources < 32768            (needs AG r1)
CH_H = 7                    # sources >= HI_BASE         (needs AG r2)
CAP_A = CH_A * P            # 640
CAP_B = CH_B * P
CAP_H = CH_H * P            # 896
CPT = CH_A + CH_B + CH_H    # 17 gathered chunks per tile
NCHUNK = CPT + 1            # + self-diagonal chunk in B
EDGE_CAP = CPT * P          # 2176 non-self edges per tile

# per-group gidx columns (idx wrapped 16-wide); per tile: 40/40/56 cols
TCOL_A = CAP_A // 16        # 40
TCOL_B = CAP_B // 16
TCOL_H = CAP_H // 16        # 56
TCOLS = TCOL_A + TCOL_B + TCOL_H               # 136 per tile
GIDX_COLS = TILES * TCOLS                      # 6664

# pool depths (groups)
GBUF_A = 4
GBUF_B = 4
GBUF_H = 4
BSL = 6                     # streamed odd-tile B slices per class pool
SLICE_AHEAD = 3             # groups

XG = 4                      # xt tiles per DMA group
NXG = (TILES + XG - 1) // XG

TRACE = False
LAST_EXEC_NS = None

_PROGRAM = None


def tiles_in(grp):
    return min(GRP, TILES - GRP * grp)


def _build_program():
    nc = bacc.Bacc(None, target_bir_lowering=False, num_swdge_queues=4)
    f32 = mybir.dt.float32
    bf16 = mybir.dt.bfloat16
    fp8 = mybir.dt.float8e4

    xt_d = nc.dram_tensor("xt", [NXG, P, XG * 4 * D], bf16,
                          kind="ExternalInput")
    w1_d = nc.dram_tensor("w1", [FIN, D], bf16, kind="ExternalInput")
    w2_d = nc.dram_tensor("w2", [D, D], bf16, kind="ExternalInput")
    wl_d = nc.dram_tensor("wl", [D, NCLS], bf16, kind="ExternalInput")
    b1_d = nc.dram_tensor("b1", [P, 1], f32, kind="ExternalInput")
    b2_d = nc.dram_tensor("b2", [P, 1], f32, kind="ExternalInput")
    bl_d = nc.dram_tensor("bl", [P, NCLS], f32, kind="ExternalInput")
    gidx_d = nc.dram_tensor("gidx", [P, GIDX_COLS], mybir.dt.int16,
                            kind="ExternalInput")
    bval_d = nc.dram_tensor("bval", [TILES, P, NCHUNK * P], fp8,
                            kind="ExternalInput")
    dslot_d = nc.dram_tensor("dslot", [P, TILES], f32, kind="ExternalInput")
    out_d = nc.dram_tensor("probs", [NLOC, NCLS], f32, kind="ExternalOutput")

    with tile.TileContext(nc) as tc:
        lib = nc.gpsimd.load_library(mlp)
        first_gather = [True]
        qctr = [0]
        ni_regs = {n: nc.gpsimd.to_reg(n)
                   for n in (CAP_A, GRP * CAP_A, CAP_H, GRP * CAP_H)}

        from contextlib import ExitStack
        with ExitStack() as stack:
            ep = stack.enter_context
            cp = ep(tc.tile_pool(name="const", bufs=1))
            bpr = ep(tc.tile_pool(name="bres", bufs=1))
            bpa = ep(tc.tile_pool(name="bsla", bufs=BSL))
            bpb = ep(tc.tile_pool(name="bslb", bufs=BSL))
            bph = ep(tc.tile_pool(name="bslh", bufs=BSL))
            glap = ep(tc.tile_pool(name="gla", bufs=GBUF_A))
            glbp = ep(tc.tile_pool(name="glb", bufs=GBUF_B))
            ghip = ep(tc.tile_pool(name="ghi", bufs=GBUF_H))
            up = ep(tc.tile_pool(name="upool", bufs=3))
            hp = ep(tc.tile_pool(name="hpool", bufs=3))
            hdp = ep(tc.tile_pool(name="headp", bufs=3))
            xwps = ep(tc.tile_pool(name="xwps", bufs=2, space="PSUM"))
            aggps = ep(tc.tile_pool(name="aggps", bufs=4, space="PSUM"))
            dr1 = ep(tc.tile_pool(name="dram1", bufs=1, space="DRAM"))
            dr2 = ep(tc.tile_pool(name="dram2", bufs=1, space="DRAM"))
            dr3 = ep(tc.tile_pool(name="dram3", bufs=1, space="DRAM"))
            dr4 = ep(tc.tile_pool(name="dram4", bufs=1, space="DRAM"))
            # ---- constants to SBUF ----
            w1_sb = cp.tile([P, 4 * D], bf16)
            for k in range(4):
                nc.sync.dma_start(w1_sb[:, k * D:(k + 1) * D],
                                  w1_d[k * P:(k + 1) * P, :])
            w2_sb = cp.tile([P, D], bf16)
            nc.sync.dma_start(w2_sb[:], w2_d[:])
            wl_sb = cp.tile([P, NCLS], bf16)
            nc.sync.dma_start(wl_sb[:], wl_d[:])
            b1_sb = cp.tile([P, 1], f32)
            nc.sync.dma_start(b1_sb[:], b1_d[:])
            b2_sb = cp.tile([P, 1], f32)
            nc.sync.dma_start(b2_sb[:], b2_d[:])
            bl_sb = cp.tile([P, NCLS], f32)
            nc.sync.dma_start(bl_sb[:], bl_d[:])
            dslot_sb = cp.tile([P, TILES], f32)
            nc.sync.dma_start(dslot_sb[:], dslot_d[:])
            gidx_sb = cp.tile([P, GIDX_COLS], mybir.dt.int16)
            nc.scalar.dma_start(gidx_sb[:], gidx_d[:])

            # resident XW outputs (layer li overwrites in place per tile)
            t_all = cp.tile([P, TILES * D], bf16)
            # SBUF f32 accumulator per dest tile (overwritten per layer)
            acc_all = cp.tile([P, TILES * P], f32)

            t_loc = [dr1.tile([NLOC, D], bf16, name="t_loc0"),
                     dr2.tile([NLOC, D], bf16, name="t_loc1")]
            t_full = [dr3.tile([VTOT, D], bf16, name="t_full0"),
                      dr4.tile([VTOT, D], bf16, name="t_full1")]
            b_res = {t: bpr.tile([P, NCHUNK * P], fp8, name=f"bres{t}")
                     for t in range(0, TILES, 2)}
            for t in range(0, TILES, 2):
                nc.scalar.dma_start(b_res[t][:], bval_d[t, :, :])

            def cc_ag(li, r):
                a, b = RT[r]
                nc.gpsimd.collective_compute(
                    "AllGather", mybir.AluOpType.bypass,
                    replica_groups=[list(range(NCORES))],
                    ins=[t_loc[li][a * P:b * P, :].opt()],
                    outs=[t_full[li][RBASE[r]:RBASE[r] + NCORES * RROWS[r],
                                     :].opt()],
                )

            # ---- phase 0: XW1 (table rows scaled by dis[src]) ----
            with tc.tile_pool(name="xtp", bufs=2) as xtp:
                for g in range(NXG):
                    xtt = xtp.tile([P, XG * 4 * D], bf16, tag="xt")
                    nc.sync.dma_start(xtt[:], xt_d[g, :, :])
                    for ti in range(XG):
                        t = g * XG + ti
                        if t >= TILES:
                            break
                        ps = xwps.tile([P, D], f32, tag="xw")
                        for k in range(4):
                            nc.tensor.matmul(
                                out=ps[:],
                                lhsT=xtt[:, (ti * 4 + k) * D:(ti * 4 + k + 1) * D],
                                rhs=w1_sb[:, k * D:(k + 1) * D],
                                start=(k == 0), stop=(k == 3),
                            )
                        nc.scalar.activation(
                            out=t_all[:, t * D:(t + 1) * D], in_=ps[:],
                            func=mybir.ActivationFunctionType.Copy,
                            scale=dslot_sb[:, t:t + 1])
                        nc.sync.dma_start(t_loc[0][t * P:(t + 1) * P, :],
                                          t_all[:, t * D:(t + 1) * D])
                        if t == RT[0][1] - 1:
                            cc_ag(0, 0)
                        elif t == RT[1][1] - 1:
                            cc_ag(0, 1)
                        elif t == RT[2][1] - 1:
                            cc_ag(0, 2)

            # ---- gather + B-slice machinery ----
            CLS = {
                "a": dict(pool=glap, bpool=bpa, cap=CAP_A, ch=CH_A,
                          src0=0, src1=16384, tcoff=0, ch0=0, bch=CH_A),
                "b": dict(pool=glbp, bpool=bpb, cap=CAP_B, ch=CH_B,
                          src0=0, src1=32768, tcoff=TCOL_A, ch0=CH_A,
                          bch=CH_B),
                "h": dict(pool=ghip, bpool=bph, cap=CAP_H, ch=CH_H,
                          src0=HI_BASE, src1=VTOT, tcoff=TCOL_A + TCOL_B,
                          ch0=CH_A + CH_B, bch=CH_H + 1),
            }
            # gidx layout: per group g: per class: tiles of the group
            # contiguous.  Column base for (g, class) precomputed:
            GCOL = {}
            col = 0
            for g_ in range(NGRP):
                nt_ = tiles_in(g_)
                for cn_, w_ in (("a", TCOL_A), ("b", TCOL_B), ("h", TCOL_H)):
                    GCOL[(g_, cn_)] = col
                    col += nt_ * w_
            assert col == GIDX_COLS

            g_buf = {}
            bsl_buf = {}

            def issue_gather(li, grp, cname):
                c = CLS[cname]
                nt = tiles_in(grp)
                ni = nt * c["cap"]
                g = c["pool"].tile([P, GRP * c["ch"] * D], bf16, tag=cname,
                                   name=f"g{cname}{li}_{grp}")
                col0 = GCOL[(grp, cname)]
                src = t_full[li][c["src0"]:c["src1"], :]
                qn = qctr[0] % 4
                qctr[0] += 1
                gi = nc.gpsimd.dma_gather(
                    g[:, :ni // P * D].rearrange("p (c d) -> p c d", d=D),
                    src,
                    gidx_sb[:, col0:col0 + ni // 16],
                    ni, ni_regs[ni], D, single_packet=False,
                    queue_num=qn,
                )
                if first_gather[0]:
                    add_dep_helper(gi.ins, lib.ins, reason="lib before gather")
                    first_gather[0] = False
                g_buf[(li, grp, cname)] = g

            # B slices for odd tiles: JIT stream on the scalar queue.
            slice_seq = [(li, cname, grp)
                         for li in (0, 1)
                         for cname in ("a", "b", "h")
                         for grp in range(NGRP)]
            slice_ptr = [0]

            def pump_slices(upto):
                while slice_ptr[0] < min(upto, len(slice_seq)):
                    li, cname, grp = slice_seq[slice_ptr[0]]
                    slice_ptr[0] += 1
                    c = CLS[cname]
                    tiles = []
                    for ti in range(tiles_in(grp)):
                        t = GRP * grp + ti
                        if t % 2 == 0:
                            tiles.append((b_res[t], c["ch0"]))
                        else:
                            bt = c["bpool"].tile([P, c["bch"] * P], fp8,
                                                 tag=cname)
                            nc.scalar.dma_start(
                                bt[:],
                                bval_d[t, :, c["ch0"] * P:
                                       (c["ch0"] + c["bch"]) * P])
                            tiles.append((bt, 0))
                    bsl_buf[(li, cname, grp)] = tiles

            def phase_ab(li, grp, cname):
                """class-chunk matmul group -> acc (copy for a, += for b)."""
                c = CLS[cname]
                g = g_buf.pop((li, grp, cname))
                btl = bsl_buf.pop((li, cname, grp))
                nch = c["ch"]
                for ti in range(tiles_in(grp)):
                    t = GRP * grp + ti
                    bt, bc0 = btl[ti]
                    ps = aggps.tile([P, P], f32, tag="agg")
                    for k in range(nch):
                        nc.tensor.matmul(
                            out=ps[:],
                            lhsT=g[:, (ti * nch + k) * D:
                                   (ti * nch + k + 1) * D],
                            rhs=bt[:, (bc0 + k) * P:(bc0 + k + 1) * P],
                            start=(k == 0), stop=(k == nch - 1),
                        )
                    acc = acc_all[:, t * P:(t + 1) * P]
                    if cname == "a":
                        nc.scalar.activation(
                            out=acc, in_=ps[:],
                            func=mybir.ActivationFunctionType.Copy)
                    else:
                        nc.vector.tensor_add(out=acc, in0=acc, in1=ps[:])

            def phase_h(li, grp):
                """hi+self matmul group, then finish tile: relu, XW2/head."""
                g = g_buf.pop((li, grp, "h"))
                btl = bsl_buf.pop((li, "h", grp))
                for ti in range(tiles_in(grp)):
                    t = GRP * grp + ti
                    bt, bc0 = btl[ti]
                    ps = aggps.tile([P, P], f32, tag="agg")
                    for k in range(CH_H):
                        nc.tensor.matmul(
                            out=ps[:],
                            lhsT=g[:, (ti * CH_H + k) * D:
                                   (ti * CH_H + k + 1) * D],
                            rhs=bt[:, (bc0 + k) * P:(bc0 + k + 1) * P],
                            start=(k == 0), stop=False,
                        )
                    # self-diagonal chunk (2*dis_dst) against resident XW
                    nc.tensor.matmul(
                        out=ps[:],
                        lhsT=t_all[:, t * D:(t + 1) * D],
                        rhs=bt[:, (bc0 + CH_H) * P:(bc0 + CH_H + 1) * P],
                        start=False, stop=True,
                    )
                    u = up.tile([P, P], f32, tag="u")
                    nc.vector.tensor_add(out=u[:],
                                         in0=acc_all[:, t * P:(t + 1) * P],
                                         in1=ps[:])
                    bias_sb = b1_sb if li == 0 else b2_sb
                    h = hp.tile([P, P], bf16, tag="h")
                    nc.scalar.activation(out=h[:], in_=u[:],
                                         func=mybir.ActivationFunctionType.Relu,
                                         bias=bias_sb[:])
                    if li == 0:
                        ps2 = xwps.tile([P, D], f32, tag="xw")
                        nc.tensor.matmul(out=ps2[:], lhsT=h[:], rhs=w2_sb[:],
                                         start=True, stop=True)
                        nc.scalar.activation(
                            out=t_all[:, t * D:(t + 1) * D], in_=ps2[:],
                            func=mybir.ActivationFunctionType.Copy,
                            scale=dslot_sb[:, t:t + 1])
                        nc.sync.dma_start(t_loc[1][t * P:(t + 1) * P, :],
                                          t_all[:, t * D:(t + 1) * D])
                        if t == RT[0][1] - 1:
                            cc_ag(1, 0)
                        elif t == RT[1][1] - 1:
                            cc_ag(1, 1)
                        elif t == RT[2][1] - 1:
                            cc_ag(1, 2)
                    else:
                        lg = xwps.tile([P, NCLS], f32, tag="xw")
                        nc.tensor.matmul(out=lg[:], lhsT=h[:], rhs=wl_sb[:],
                                         start=True, stop=True)
                        l_sb = hdp.tile([P, NCLS], f32, tag="l")
                        nc.vector.tensor_add(out=l_sb[:], in0=lg[:],
                                             in1=bl_sb[:])
                        nmx = hdp.tile([P, 1], f32, tag="nmx")
                        nc.vector.reduce_max(out=nmx[:], in_=l_sb[:],
                                             axis=mybir.AxisListType.X,
                                             negate=True)
                        e_sb = hdp.tile([P, NCLS], f32, tag="e")
                        nc.scalar.activation(
                            out=e_sb[:], in_=l_sb[:],
                            func=mybir.ActivationFunctionType.Exp,
                            bias=nmx[:])
                        sm = hdp.tile([P, 1], f32, tag="sm")
                        nc.vector.reduce_sum(out=sm[:], in_=e_sb[:],
                                             axis=mybir.AxisListType.X)
                        rs = hdp.tile([P, 1], f32, tag="rs")
                        nc.vector.reciprocal(out=rs[:], in_=sm[:])
                        pr = hdp.tile([P, NCLS], f32, tag="pr")
                        nc.scalar.activation(
                            out=pr[:], in_=e_sb[:],
                            func=mybir.ActivationFunctionType.Copy,
                            scale=rs[:])
                        nc.sync.dma_start(out_d[t * P:(t + 1) * P, :],
                                          pr[:])

            # ---- emit both layers ----
            # gpsimd issue order (in-order dispatch!):
            #   L1: a x13, b x13, h g0..5, AGtrig(1,0) [emitted inside
            #   phase_h(g3) at t==15], h g6..9, AG(1,1) [phase_h(g7)],
            #   h g10..12, AG(1,2) [phase_h(g12)], then L2 a, b, h.
            # The triggers are emitted by the phase loop; the gather issues
            # are deferred accordingly so the triggers land between them.
            for cname in ("a", "b"):
                for grp in range(NGRP):
                    issue_gather(0, grp, cname)
            for grp in range(6):
                issue_gather(0, grp, "h")
            pump_slices(SLICE_AHEAD)

            G = [0]

            def step():
                G[0] += 1
                pump_slices(G[0] + SLICE_AHEAD)

            for li in (0, 1):
                for cname in ("a", "b"):
                    for grp in range(NGRP):
                        phase_ab(li, grp, cname)
                        step()
                for grp in range(NGRP):
                    phase_h(li, grp)
                    step()
                    if li == 0:
                        if grp == 3:      # cc_ag(1,0) just emitted (t==15)
                            for g2 in range(6, 10):
                                issue_gather(0, g2, "h")
                        elif grp == 7:    # cc_ag(1,1) just emitted (t==31)
                            for g2 in range(10, NGRP):
                                issue_gather(0, g2, "h")
                        elif grp == NGRP - 1:  # cc_ag(1,2) emitted (t==48)
                            for cname in ("a", "b", "h"):
                                for g2 in range(NGRP):
                                    issue_gather(1, g2, cname)
                # emit the layer-1 AG triggers inside phase_h via t checks
                # (handled in phase_h's t_loc write section below)

    nc.compile()
    return nc


def _preprocess(x, edge_index, W1, b1, W2, b2, Wlin, blin):
    """Host-side graph preprocessing -> per-core input dicts + slot maps."""
    fp8np = mybir.dt.np(mybir.dt.float8e4)
    x = np.asarray(x, np.float32)
    ei = np.asarray(edge_index)
    row = ei[0].astype(np.int64)
    col = ei[1].astype(np.int64)

    deg = np.bincount(col, minlength=N).astype(np.float32) + 2.0
    dis = (1.0 / np.sqrt(deg)).astype(np.float32)

    indeg = np.bincount(col, minlength=N)  # non-self in-edges

    # balanced node->bin assignment (bins = core*TILES + tile), snake by degree
    NB = NCORES * TILES
    order = np.argsort(-indeg, kind="stable")
    bin_of_node = np.empty(N, np.int64)
    pos_in_bin = np.empty(N, np.int64)
    full_rounds = N // NB
    rem = N - full_rounds * NB
    fwd = np.arange(NB)
    bwd = fwd[::-1]
    seq = []
    for r in range(full_rounds):
        seq.append(fwd if r % 2 == 0 else bwd)
    if rem:
        seq.append((fwd if full_rounds % 2 == 0 else bwd)[:rem])
    seq = np.concatenate(seq)
    bin_of_node[order] = seq
    srt = np.argsort(bin_of_node, kind="stable")
    cnt = np.bincount(bin_of_node, minlength=NB)
    assert cnt.max() <= P
    starts = np.zeros(NB + 1, np.int64)
    np.cumsum(cnt, out=starts[1:])
    pos_in_bin[srt] = np.arange(N) - starts[bin_of_node[srt]]

    bin_edge_cnt = np.bincount(bin_of_node[col], minlength=NB)
    assert bin_edge_cnt.max() <= EDGE_CAP, (
        f"bin edge overflow: {bin_edge_cnt.max()} > {EDGE_CAP}")

    core_of_node = bin_of_node // TILES
    tile_of_node = bin_of_node % TILES
    # region-major table row
    region_of_tile = np.where(tile_of_node < RT[0][1], 0,
                              np.where(tile_of_node < RT[1][1], 1, 2))
    rbase = np.array(RBASE, np.int64)[region_of_tile]
    rrows = np.array(RROWS, np.int64)[region_of_tile]
    rt0 = np.array([RT[0][0], RT[1][0], RT[2][0]], np.int64)[region_of_tile]
    gslot = (rbase + core_of_node * rrows
             + (tile_of_node - rt0) * P + pos_in_bin)

    # per-edge: destination bin + dest position + dest dis; source table slot
    e_bin = bin_of_node[col]
    e_dpos = pos_in_bin[col]
    e_src = gslot[row]
    e_dis = dis[col]

    e_order = np.argsort(e_bin, kind="stable")
    eb = e_bin[e_order]
    ed = e_dpos[e_order]
    es = e_src[e_order]
    ew = e_dis[e_order]
    bstarts = np.searchsorted(eb, np.arange(NB + 1))

    in_maps = []
    caps = (CAP_A, CAP_B, CAP_H)
    bases = (0, 0, HI_BASE)
    ch0s = (0, CH_A, CH_A + CH_B)
    tcols = (TCOL_A, TCOL_B, TCOL_H)
    for c in range(NCORES):
        gidx = np.zeros((P, GIDX_COLS), np.int16)
        bval = np.zeros((TILES, P, NCHUNK * P), fp8np)
        cls_idx = np.zeros((TILES, 3, CAP_H), np.int64)
        cls_n = np.zeros((TILES, 3), np.int64)
        for t in range(TILES):
            b = c * TILES + t
            lo_f, hi_f = bstarts[b], bstarts[b + 1]
            o = np.argsort(es[lo_f:hi_f], kind="stable")
            srcs = es[lo_f:hi_f][o]
            dpos_s = ed[lo_f:hi_f][o]
            wdis_s = ew[lo_f:hi_f][o]
            ne = len(srcs)
            lo_elig = int((srcs < 32768).sum())
            n_a = min(int((srcs < 16384).sum()), CAP_A)
            n_b = min(lo_elig - n_a, CAP_B)
            rest = ne - n_a - n_b
            assert rest <= CAP_H, f"hi overflow t={t} c={c}: {rest}"
            if rest:
                assert srcs[n_a + n_b] >= HI_BASE, (
                    f"hi source below HI_BASE: {srcs[n_a + n_b]}")
            parts = (slice(0, n_a), slice(n_a, n_a + n_b),
                     slice(n_a + n_b, ne))
            for k in range(3):
                sl = parts[k]
                kk = sl.stop - sl.start
                cls_n[t, k] = kk
                cls_idx[t, k, :kk] = srcs[sl] - bases[k]
                ii = np.arange(kk)
                cidx = ch0s[k] + ii // P
                pidx = ii % P
                bval[t, pidx, cidx * P + dpos_s[sl]] = (
                    wdis_s[sl].astype(fp8np))
        # self-diagonal chunk: 2*dis at (slot, chunk NCHUNK-1, col=slot)
        mine = np.where(core_of_node == c)[0]
        tsel = tile_of_node[mine]
        psel = pos_in_bin[mine]
        bval[tsel, psel, (NCHUNK - 1) * P + psel] = (
            (2.0 * dis[mine]).astype(fp8np))

        # gidx blocks per group: [grp: A(t0..t3), B(t0..t3), H(t0..t3)]
        col0 = 0
        for grp in range(NGRP):
            nt = tiles_in(grp)
            for k in range(3):
                cap = caps[k]
                # pads gather safe index 0 (B columns there are zero); the
                # ucode num_idxs register must equal count(idx >= 0)
                flat = np.zeros(nt * cap, np.int64)
                for ti in range(nt):
                    t = GRP * grp + ti
                    kk = int(cls_n[t, k])
                    flat[ti * cap:ti * cap + kk] = cls_idx[t, k, :kk]
                w = flat.reshape(len(flat) // 16, 16).T.astype(np.int16)
                gidx[:, col0:col0 + len(flat) // 16] = np.tile(w, (8, 1))
                col0 += nt * tcols[k]
        assert col0 == GIDX_COLS

        # x slice, transposed, padded, then per-tile blocks [t, p, k*128+j]
        xt = np.zeros((FIN, NLOC), ml_dtypes.bfloat16)
        lslot = tile_of_node[mine] * P + pos_in_bin[mine]
        xt[:, lslot] = x[mine].T.astype(ml_dtypes.bfloat16)
        xtr = np.zeros((NXG * XG, P, 4 * D), ml_dtypes.bfloat16)
        xtr[:TILES] = np.ascontiguousarray(
            xt.reshape(4, P, TILES, P).transpose(2, 1, 0, 3)
        ).reshape(TILES, P, 4 * D)
        xtr = np.ascontiguousarray(
            xtr.reshape(NXG, XG, P, 4 * D).transpose(0, 2, 1, 3)
        ).reshape(NXG, P, XG * 4 * D)
        # per-slot dis (for table scaling)
        dslot = np.zeros((P, TILES), np.float32)
        dslot[lslot % P, lslot // P] = dis[mine]
        in_maps.append({
            "xt": xtr,
            "w1": np.asarray(W1).astype(ml_dtypes.bfloat16),
            "w2": np.asarray(W2).astype(ml_dtypes.bfloat16),
            "wl": np.asarray(Wlin).astype(ml_dtypes.bfloat16),
            "b1": np.asarray(b1, np.float32).reshape(P, 1),
            "b2": np.asarray(b2, np.float32).reshape(P, 1),
            "bl": np.tile(np.asarray(blin, np.float32).reshape(1, NCLS), (P, 1)),
            "gidx": gidx,
            "bval": bval,
            "dslot": dslot,
        })
    return in_maps, core_of_node, tile_of_node, pos_in_bin


def kernel(x, edge_index, W1, b1, W2, b2, Wlin, blin):
    global _PROGRAM, LAST_EXEC_NS
    in_maps, core_of, tile_of, pos_of = _preprocess(
        x, edge_index, W1, b1, W2, b2, Wlin, blin)
    if _PROGRAM is None:
        _PROGRAM = _build_program()
    res = run_bass_kernel_spmd(
        _PROGRAM, in_maps, core_ids=list(range(NCORES)), trace=TRACE)
    LAST_EXEC_NS = res.exec_time_ns
    out = np.empty((N, NCLS), np.float32)
    per_core = [res.results[c]["probs"] for c in range(NCORES)]
    lslot = tile_of * P + pos_of
    for c in range(NCORES):
        mine = np.where(core_of == c)[0]
        out[mine] = per_core[c][lslot[mine]]
    return out


# revision 17
# speedup vs baseline: 1.1204x; 1.1204x over previous
"""2-layer GCN (improved=True) + linear head + softmax on 8 Trainium2 cores.

Strategy (dest-node partitioning, v4: class-phased gather pipeline,
4-tile gather groups):
- Nodes assigned to 8 cores x 49 tiles x 128 slots via balanced bin-packing.
  Self-loops are never gathered: every XW output tile stays resident in SBUF
  (t_all) and the self term is one extra matmul against a per-tile diagonal
  (2*dis_dst) kept in the B matrix.
- Normalization fully folded into data: table rows are dis_src * XW (scaled
  at the PSUM->SBUF copy), the one-hot scatter matrix B holds fp8(dis_dst),
  so aggregation needs no post-scaling at all.
- Table is region-major: region 0 = tiles 0..15 (rows 0:16384), region 1 =
  tiles 16..31 (16384:32768), region 2 = tiles 32..48 (32768:50176).  Each
  layer runs THREE AllGathers (one per region).  Triggers are interleaved
  into the gpsimd gather-issue order mid-stream so they dispatch promptly
  (the gpsimd engine is in-order) while their semaphore deps keep them
  exact.
- Per dest tile, edges go to 3 gather classes with fixed chunk budgets:
  lo-a (5 chunks, sources < 16384, dep AG r0), lo-b (5 chunks, sources <
  32768, dep AG r1), hi (7 chunks, sources >= 17408, dep AG r2).  Gathers
  cover FOUR tiles per instruction (amortizing per-instruction overhead)
  on a strict 4-queue SWDGE rotation, issued class-major.
- Compute is phased per layer: all lo-a chunk matmul groups first
  (recycling gather buffers while AG r1/r2 are in flight), then all lo-b
  groups, then per-tile hi+self groups plus the nonlinearity and the
  next-layer XW.  Each class group drains PSUM into an SBUF f32
  accumulator (acc_all), so only a few PSUM banks are ever live.
- B matrices: even tiles resident in SBUF; odd tiles streamed per class
  with a small lookahead on the scalar queue.
- Head: logits = H2 @ Wlin + blin, softmax over 8 classes.

kernel() is self-contained: host-side numpy does all graph preprocessing;
the device program is identical on all 8 cores, only data differs.
"""
import sys

sys.path.insert(0, "/opt/trn_rl_repo")

import numpy as np
import ml_dtypes

import concourse.bass as bass
import concourse.bacc as bacc
import concourse.mybir as mybir
import concourse.tile as tile
from concourse.tile_rust import add_dep_helper
from concourse.bass_utils import run_bass_kernel_spmd
from concourse.library_config import mlp

# problem constants
N = 50000
E = 800000
FIN = 512
D = 128
NCLS = 8
NCORES = 8

# sharding constants
P = 128
TILES = 49
NLOC = TILES * P            # 6272 slots per core
VTOT = NCORES * NLOC        # 50176 table rows

GRP = 2                     # tiles per gather group
NGRP = (TILES + GRP - 1) // GRP     # 13 (last group has 1 tile)

# region-major table: region r holds tiles RT[r][0]..RT[r][1] of every core
RT = [(0, 16), (16, 32), (32, TILES)]
RROWS = [(b - a) * P for a, b in RT]           # rows per core per region
RBASE = [0, 16384, 32768]                      # global row base per region
HI_BASE = 17408                                # hi gathers read table[HI_BASE:]

# gather classes: chunks per tile
CH_A = 5                    # sources < 16384            (needs AG r0)
CH_B = 5                    # sources < 32768            (needs AG r1)
CH_H = 7                    # sources >= HI_BASE         (needs AG r2)
CAP_A = CH_A * P            # 640
CAP_B = CH_B * P
CAP_H = CH_H * P            # 896
CPT = CH_A + CH_B + CH_H    # 17 gathered chunks per tile
NCHUNK = CPT + 1            # + self-diagonal chunk in B
EDGE_CAP = CPT * P          # 2176 non-self edges per tile

# per-group gidx columns (idx wrapped 16-wide); per tile: 40/40/56 cols
TCOL_A = CAP_A // 16        # 40
TCOL_B = CAP_B // 16
TCOL_H = CAP_H // 16        # 56
TCOLS = TCOL_A + TCOL_B + TCOL_H               # 136 per tile
GIDX_COLS = TILES * TCOLS                      # 6664

# pool depths (groups)
GBUF_A = 8
GBUF_B = 8
GBUF_H = 8
BSL = 8                     # streamed odd-tile B slices per class pool
SLICE_AHEAD = 6             # groups

XG = 4                      # xt tiles per DMA group
NXG = (TILES + XG - 1) // XG

TRACE = False
LAST_EXEC_NS = None

_PROGRAM = None


def tiles_in(grp):
    return min(GRP, TILES - GRP * grp)


def _build_program():
    nc = bacc.Bacc(None, target_bir_lowering=False, num_swdge_queues=4)
    f32 = mybir.dt.float32
    bf16 = mybir.dt.bfloat16
    fp8 = mybir.dt.float8e4

    xt_d = nc.dram_tensor("xt", [NXG, P, XG * 4 * D], bf16,
                          kind="ExternalInput")
    w1_d = nc.dram_tensor("w1", [FIN, D], bf16, kind="ExternalInput")
    w2_d = nc.dram_tensor("w2", [D, D], bf16, kind="ExternalInput")
    wl_d = nc.dram_tensor("wl", [D, NCLS], bf16, kind="ExternalInput")
    b1_d = nc.dram_tensor("b1", [P, 1], f32, kind="ExternalInput")
    b2_d = nc.dram_tensor("b2", [P, 1], f32, kind="ExternalInput")
    bl_d = nc.dram_tensor("bl", [P, NCLS], f32, kind="ExternalInput")
    gidx_d = nc.dram_tensor("gidx", [P, GIDX_COLS], mybir.dt.int16,
                            kind="ExternalInput")
    bval_d = nc.dram_tensor("bval", [TILES, P, NCHUNK * P], fp8,
                            kind="ExternalInput")
    dslot_d = nc.dram_tensor("dslot", [P, TILES], f32, kind="ExternalInput")
    out_d = nc.dram_tensor("probs", [NLOC, NCLS], f32, kind="ExternalOutput")

    with tile.TileContext(nc) as tc:
        lib = nc.gpsimd.load_library(mlp)
        first_gather = [True]
        qctr = [0]
        ni_regs = {n: nc.gpsimd.to_reg(n)
                   for n in (CAP_A, GRP * CAP_A, CAP_H, GRP * CAP_H)}

        from contextlib import ExitStack
        with ExitStack() as stack:
            ep = stack.enter_context
            cp = ep(tc.tile_pool(name="const", bufs=1))
            bpr = ep(tc.tile_pool(name="bres", bufs=1))
            bpa = ep(tc.tile_pool(name="bsla", bufs=BSL))
            bpb = ep(tc.tile_pool(name="bslb", bufs=BSL))
            bph = ep(tc.tile_pool(name="bslh", bufs=BSL))
            glap = ep(tc.tile_pool(name="gla", bufs=GBUF_A))
            glbp = ep(tc.tile_pool(name="glb", bufs=GBUF_B))
            ghip = ep(tc.tile_pool(name="ghi", bufs=GBUF_H))
            up = ep(tc.tile_pool(name="upool", bufs=4))
            hp = ep(tc.tile_pool(name="hpool", bufs=6))
            hdp = ep(tc.tile_pool(name="headp", bufs=3))
            xwps = ep(tc.tile_pool(name="xwps", bufs=2, space="PSUM"))
            aggps = ep(tc.tile_pool(name="aggps", bufs=6, space="PSUM"))
            dr1 = ep(tc.tile_pool(name="dram1", bufs=1, space="DRAM"))
            dr2 = ep(tc.tile_pool(name="dram2", bufs=1, space="DRAM"))
            dr3 = ep(tc.tile_pool(name="dram3", bufs=1, space="DRAM"))
            dr4 = ep(tc.tile_pool(name="dram4", bufs=1, space="DRAM"))
            # ---- constants to SBUF ----
            w1_sb = cp.tile([P, 4 * D], bf16)
            for k in range(4):
                nc.sync.dma_start(w1_sb[:, k * D:(k + 1) * D],
                                  w1_d[k * P:(k + 1) * P, :])
            w2_sb = cp.tile([P, D], bf16)
            nc.sync.dma_start(w2_sb[:], w2_d[:])
            wl_sb = cp.tile([P, NCLS], bf16)
            nc.sync.dma_start(wl_sb[:], wl_d[:])
            b1_sb = cp.tile([P, 1], f32)
            nc.sync.dma_start(b1_sb[:], b1_d[:])
            b2_sb = cp.tile([P, 1], f32)
            nc.sync.dma_start(b2_sb[:], b2_d[:])
            bl_sb = cp.tile([P, NCLS], f32)
            nc.sync.dma_start(bl_sb[:], bl_d[:])
            dslot_sb = cp.tile([P, TILES], f32)
            nc.sync.dma_start(dslot_sb[:], dslot_d[:])
            gidx_sb = cp.tile([P, GIDX_COLS], mybir.dt.int16)
            nc.scalar.dma_start(gidx_sb[:], gidx_d[:])

            # resident XW outputs (layer li overwrites in place per tile)
            t_all = cp.tile([P, TILES * D], bf16)
            # SBUF f32 accumulator per dest tile (overwritten per layer)
            acc_all = cp.tile([P, TILES * P], f32)

            t_loc = [dr1.tile([NLOC, D], bf16, name="t_loc0"),
                     dr2.tile([NLOC, D], bf16, name="t_loc1")]
            t_full = [dr3.tile([VTOT, D], bf16, name="t_full0"),
                      dr4.tile([VTOT, D], bf16, name="t_full1")]
            b_res = {t: bpr.tile([P, NCHUNK * P], fp8, name=f"bres{t}")
                     for t in range(0, TILES, 2)}
            for t in range(0, TILES, 2):
                nc.scalar.dma_start(b_res[t][:], bval_d[t, :, :])

            def cc_ag(li, r):
                a, b = RT[r]
                nc.gpsimd.collective_compute(
                    "AllGather", mybir.AluOpType.bypass,
                    replica_groups=[list(range(NCORES))],
                    ins=[t_loc[li][a * P:b * P, :].opt()],
                    outs=[t_full[li][RBASE[r]:RBASE[r] + NCORES * RROWS[r],
                                     :].opt()],
                )

            # ---- phase 0: XW1 (table rows scaled by dis[src]) ----
            with tc.tile_pool(name="xtp", bufs=2) as xtp:
                for g in range(NXG):
                    xtt = xtp.tile([P, XG * 4 * D], bf16, tag="xt")
                    nc.sync.dma_start(xtt[:], xt_d[g, :, :])
                    for ti in range(XG):
                        t = g * XG + ti
                        if t >= TILES:
                            break
                        ps = xwps.tile([P, D], f32, tag="xw")
                        for k in range(4):
                            nc.tensor.matmul(
                                out=ps[:],
                                lhsT=xtt[:, (ti * 4 + k) * D:(ti * 4 + k + 1) * D],
                                rhs=w1_sb[:, k * D:(k + 1) * D],
                                start=(k == 0), stop=(k == 3),
                            )
                        nc.scalar.activation(
                            out=t_all[:, t * D:(t + 1) * D], in_=ps[:],
                            func=mybir.ActivationFunctionType.Copy,
                            scale=dslot_sb[:, t:t + 1])
                        nc.sync.dma_start(t_loc[0][t * P:(t + 1) * P, :],
                                          t_all[:, t * D:(t + 1) * D])
                        if t == RT[0][1] - 1:
                            cc_ag(0, 0)
                        elif t == RT[1][1] - 1:
                            cc_ag(0, 1)
                        elif t == RT[2][1] - 1:
                            cc_ag(0, 2)

            # ---- gather + B-slice machinery ----
            CLS = {
                "a": dict(pool=glap, bpool=bpa, cap=CAP_A, ch=CH_A,
                          src0=0, src1=16384, tcoff=0, ch0=0, bch=CH_A),
                "b": dict(pool=glbp, bpool=bpb, cap=CAP_B, ch=CH_B,
                          src0=0, src1=32768, tcoff=TCOL_A, ch0=CH_A,
                          bch=CH_B),
                "h": dict(pool=ghip, bpool=bph, cap=CAP_H, ch=CH_H,
                          src0=HI_BASE, src1=VTOT, tcoff=TCOL_A + TCOL_B,
                          ch0=CH_A + CH_B, bch=CH_H + 1),
            }
            # gidx layout: per group g: per class: tiles of the group
            # contiguous.  Column base for (g, class) precomputed:
            GCOL = {}
            col = 0
            for g_ in range(NGRP):
                nt_ = tiles_in(g_)
                for cn_, w_ in (("a", TCOL_A), ("b", TCOL_B), ("h", TCOL_H)):
                    GCOL[(g_, cn_)] = col
                    col += nt_ * w_
            assert col == GIDX_COLS

            g_buf = {}
            bsl_buf = {}

            def issue_gather(li, grp, cname):
                c = CLS[cname]
                nt = tiles_in(grp)
                ni = nt * c["cap"]
                g = c["pool"].tile([P, GRP * c["ch"] * D], bf16, tag=cname,
                                   name=f"g{cname}{li}_{grp}")
                col0 = GCOL[(grp, cname)]
                src = t_full[li][c["src0"]:c["src1"], :]
                qn = qctr[0] % 4
                qctr[0] += 1
                gi = nc.gpsimd.dma_gather(
                    g[:, :ni // P * D].rearrange("p (c d) -> p c d", d=D),
                    src,
                    gidx_sb[:, col0:col0 + ni // 16],
                    ni, ni_regs[ni], D, single_packet=False,
                    queue_num=qn,
                )
                if first_gather[0]:
                    add_dep_helper(gi.ins, lib.ins, reason="lib before gather")
                    first_gather[0] = False
                g_buf[(li, grp, cname)] = g

            # B slices for odd tiles: JIT stream on the scalar queue.
            slice_seq = [(li, cname, grp)
                         for li in (0, 1)
                         for cname in ("a", "b", "h")
                         for grp in range(NGRP)]
            slice_ptr = [0]

            def pump_slices(upto):
                while slice_ptr[0] < min(upto, len(slice_seq)):
                    li, cname, grp = slice_seq[slice_ptr[0]]
                    slice_ptr[0] += 1
                    c = CLS[cname]
                    tiles = []
                    for ti in range(tiles_in(grp)):
                        t = GRP * grp + ti
                        if t % 2 == 0:
                            tiles.append((b_res[t], c["ch0"]))
                        else:
                            bt = c["bpool"].tile([P, c["bch"] * P], fp8,
                                                 tag=cname)
                            nc.scalar.dma_start(
                                bt[:],
                                bval_d[t, :, c["ch0"] * P:
                                       (c["ch0"] + c["bch"]) * P])
                            tiles.append((bt, 0))
                    bsl_buf[(li, cname, grp)] = tiles

            def phase_ab(li, grp, cname):
                """class-chunk matmul group -> acc (copy for a, += for b)."""
                c = CLS[cname]
                g = g_buf.pop((li, grp, cname))
                btl = bsl_buf.pop((li, cname, grp))
                nch = c["ch"]
                for ti in range(tiles_in(grp)):
                    t = GRP * grp + ti
                    bt, bc0 = btl[ti]
                    ps = aggps.tile([P, P], f32, tag="agg")
                    for k in range(nch):
                        nc.tensor.matmul(
                            out=ps[:],
                            lhsT=g[:, (ti * nch + k) * D:
                                   (ti * nch + k + 1) * D],
                            rhs=bt[:, (bc0 + k) * P:(bc0 + k + 1) * P],
                            start=(k == 0), stop=(k == nch - 1),
                        )
                    acc = acc_all[:, t * P:(t + 1) * P]
                    if cname == "a":
                        nc.scalar.activation(
                            out=acc, in_=ps[:],
                            func=mybir.ActivationFunctionType.Copy)
                    else:
                        nc.vector.tensor_add(out=acc, in0=acc, in1=ps[:])

            # phase_h is software-pipelined: the XW2 / head matmul of tile t
            # depends (via relu on scalar) on tile t's agg group, so it is
            # deferred H_LAG tiles so the tensor queue never stalls on the
            # cross-engine chain.
            H_LAG = 3
            h_pend = []

            def emit_tail(li, t, h):
                if li == 0:
                    ps2 = xwps.tile([P, D], f32, tag="xw")
                    nc.tensor.matmul(out=ps2[:], lhsT=h[:], rhs=w2_sb[:],
                                     start=True, stop=True)
                    nc.scalar.activation(
                        out=t_all[:, t * D:(t + 1) * D], in_=ps2[:],
                        func=mybir.ActivationFunctionType.Copy,
                        scale=dslot_sb[:, t:t + 1])
                    nc.sync.dma_start(t_loc[1][t * P:(t + 1) * P, :],
                                      t_all[:, t * D:(t + 1) * D])
                    if t == RT[0][1] - 1:
                        cc_ag(1, 0)
                        for g2 in range(12, 20):
                            issue_gather(0, g2, "h")
                    elif t == RT[1][1] - 1:
                        cc_ag(1, 1)
                        for g2 in range(20, NGRP):
                            issue_gather(0, g2, "h")
                    elif t == RT[2][1] - 1:
                        cc_ag(1, 2)
                        for cn2 in ("a", "b", "h"):
                            for g2 in range(NGRP):
                                issue_gather(1, g2, cn2)
                else:
                    lg = xwps.tile([P, NCLS], f32, tag="xw")
                    nc.tensor.matmul(out=lg[:], lhsT=h[:], rhs=wl_sb[:],
                                     start=True, stop=True)
                    l_sb = hdp.tile([P, NCLS], f32, tag="l")
                    nc.vector.tensor_add(out=l_sb[:], in0=lg[:],
                                         in1=bl_sb[:])
                    nmx = hdp.tile([P, 1], f32, tag="nmx")
                    nc.vector.reduce_max(out=nmx[:], in_=l_sb[:],
                                         axis=mybir.AxisListType.X,
                                         negate=True)
                    e_sb = hdp.tile([P, NCLS], f32, tag="e")
                    nc.scalar.activation(
                        out=e_sb[:], in_=l_sb[:],
                        func=mybir.ActivationFunctionType.Exp,
                        bias=nmx[:])
                    sm = hdp.tile([P, 1], f32, tag="sm")
                    nc.vector.reduce_sum(out=sm[:], in_=e_sb[:],
                                         axis=mybir.AxisListType.X)
                    rs = hdp.tile([P, 1], f32, tag="rs")
                    nc.vector.reciprocal(out=rs[:], in_=sm[:])
                    pr = hdp.tile([P, NCLS], f32, tag="pr")
                    nc.scalar.activation(
                        out=pr[:], in_=e_sb[:],
                        func=mybir.ActivationFunctionType.Copy,
                        scale=rs[:])
                    nc.sync.dma_start(out_d[t * P:(t + 1) * P, :],
                                      pr[:])

            def h_drain(keep):
                while len(h_pend) > keep:
                    li0, t0, h0 = h_pend.pop(0)
                    emit_tail(li0, t0, h0)

            def phase_h(li, grp):
                """hi+self matmul group + relu; tail emitted H_LAG later."""
                g = g_buf.pop((li, grp, "h"))
                btl = bsl_buf.pop((li, "h", grp))
                for ti in range(tiles_in(grp)):
                    t = GRP * grp + ti
                    bt, bc0 = btl[ti]
                    ps = aggps.tile([P, P], f32, tag="agg")
                    for k in range(CH_H):
                        nc.tensor.matmul(
                            out=ps[:],
                            lhsT=g[:, (ti * CH_H + k) * D:
                                   (ti * CH_H + k + 1) * D],
                            rhs=bt[:, (bc0 + k) * P:(bc0 + k + 1) * P],
                            start=(k == 0), stop=False,
                        )
                    # self-diagonal chunk (2*dis_dst) against resident XW
                    nc.tensor.matmul(
                        out=ps[:],
                        lhsT=t_all[:, t * D:(t + 1) * D],
                        rhs=bt[:, (bc0 + CH_H) * P:(bc0 + CH_H + 1) * P],
                        start=False, stop=True,
                    )
                    u = up.tile([P, P], f32, tag="u")
                    nc.vector.tensor_add(out=u[:],
                                         in0=acc_all[:, t * P:(t + 1) * P],
                                         in1=ps[:])
                    bias_sb = b1_sb if li == 0 else b2_sb
                    h = hp.tile([P, P], bf16, tag="h")
                    nc.scalar.activation(out=h[:], in_=u[:],
                                         func=mybir.ActivationFunctionType.Relu,
                                         bias=bias_sb[:])
                    h_pend.append((li, t, h))
                    h_drain(H_LAG)

            # ---- emit both layers ----
            # gpsimd issue order (in-order dispatch!):
            #   L1: a x13, b x13, h g0..5, AGtrig(1,0) [emitted inside
            #   phase_h(g3) at t==15], h g6..9, AG(1,1) [phase_h(g7)],
            #   h g10..12, AG(1,2) [phase_h(g12)], then L2 a, b, h.
            # The triggers are emitted by the phase loop; the gather issues
            # are deferred accordingly so the triggers land between them.
            for cname in ("a", "b"):
                for grp in range(NGRP):
                    issue_gather(0, grp, cname)
            for grp in range(12):
                issue_gather(0, grp, "h")
            pump_slices(SLICE_AHEAD)

            G = [0]

            def step():
                G[0] += 1
                pump_slices(G[0] + SLICE_AHEAD)

            for li in (0, 1):
                for cname in ("a", "b"):
                    for grp in range(NGRP):
                        phase_ab(li, grp, cname)
                        step()
                for grp in range(NGRP):
                    phase_h(li, grp)
                    step()
                h_drain(0)

    nc.compile()
    return nc


def _preprocess(x, edge_index, W1, b1, W2, b2, Wlin, blin):
    """Host-side graph preprocessing -> per-core input dicts + slot maps."""
    fp8np = mybir.dt.np(mybir.dt.float8e4)
    x = np.asarray(x, np.float32)
    ei = np.asarray(edge_index)
    row = ei[0].astype(np.int64)
    col = ei[1].astype(np.int64)

    deg = np.bincount(col, minlength=N).astype(np.float32) + 2.0
    dis = (1.0 / np.sqrt(deg)).astype(np.float32)

    indeg = np.bincount(col, minlength=N)  # non-self in-edges

    # balanced node->bin assignment (bins = core*TILES + tile), snake by degree
    NB = NCORES * TILES
    order = np.argsort(-indeg, kind="stable")
    bin_of_node = np.empty(N, np.int64)
    pos_in_bin = np.empty(N, np.int64)
    full_rounds = N // NB
    rem = N - full_rounds * NB
    fwd = np.arange(NB)
    bwd = fwd[::-1]
    seq = []
    for r in range(full_rounds):
        seq.append(fwd if r % 2 == 0 else bwd)
    if rem:
        seq.append((fwd if full_rounds % 2 == 0 else bwd)[:rem])
    seq = np.concatenate(seq)
    bin_of_node[order] = seq
    srt = np.argsort(bin_of_node, kind="stable")
    cnt = np.bincount(bin_of_node, minlength=NB)
    assert cnt.max() <= P
    starts = np.zeros(NB + 1, np.int64)
    np.cumsum(cnt, out=starts[1:])
    pos_in_bin[srt] = np.arange(N) - starts[bin_of_node[srt]]

    bin_edge_cnt = np.bincount(bin_of_node[col], minlength=NB)
    assert bin_edge_cnt.max() <= EDGE_CAP, (
        f"bin edge overflow: {bin_edge_cnt.max()} > {EDGE_CAP}")

    core_of_node = bin_of_node // TILES
    tile_of_node = bin_of_node % TILES
    # region-major table row
    region_of_tile = np.where(tile_of_node < RT[0][1], 0,
                              np.where(tile_of_node < RT[1][1], 1, 2))
    rbase = np.array(RBASE, np.int64)[region_of_tile]
    rrows = np.array(RROWS, np.int64)[region_of_tile]
    rt0 = np.array([RT[0][0], RT[1][0], RT[2][0]], np.int64)[region_of_tile]
    gslot = (rbase + core_of_node * rrows
             + (tile_of_node - rt0) * P + pos_in_bin)

    # per-edge: destination bin + dest position + dest dis; source table slot
    e_bin = bin_of_node[col]
    e_dpos = pos_in_bin[col]
    e_src = gslot[row]
    e_dis = dis[col]

    e_order = np.argsort(e_bin, kind="stable")
    eb = e_bin[e_order]
    ed = e_dpos[e_order]
    es = e_src[e_order]
    ew = e_dis[e_order]
    bstarts = np.searchsorted(eb, np.arange(NB + 1))

    in_maps = []
    caps = (CAP_A, CAP_B, CAP_H)
    bases = (0, 0, HI_BASE)
    ch0s = (0, CH_A, CH_A + CH_B)
    tcols = (TCOL_A, TCOL_B, TCOL_H)
    for c in range(NCORES):
        gidx = np.zeros((P, GIDX_COLS), np.int16)
        bval = np.zeros((TILES, P, NCHUNK * P), fp8np)
        cls_idx = np.zeros((TILES, 3, CAP_H), np.int64)
        cls_n = np.zeros((TILES, 3), np.int64)
        for t in range(TILES):
            b = c * TILES + t
            lo_f, hi_f = bstarts[b], bstarts[b + 1]
            o = np.argsort(es[lo_f:hi_f], kind="stable")
            srcs = es[lo_f:hi_f][o]
            dpos_s = ed[lo_f:hi_f][o]
            wdis_s = ew[lo_f:hi_f][o]
            ne = len(srcs)
            lo_elig = int((srcs < 32768).sum())
            n_a = min(int((srcs < 16384).sum()), CAP_A)
            n_b = min(lo_elig - n_a, CAP_B)
            rest = ne - n_a - n_b
            assert rest <= CAP_H, f"hi overflow t={t} c={c}: {rest}"
            if rest:
                assert srcs[n_a + n_b] >= HI_BASE, (
                    f"hi source below HI_BASE: {srcs[n_a + n_b]}")
            parts = (slice(0, n_a), slice(n_a, n_a + n_b),
                     slice(n_a + n_b, ne))
            for k in range(3):
                sl = parts[k]
                kk = sl.stop - sl.start
                cls_n[t, k] = kk
                cls_idx[t, k, :kk] = srcs[sl] - bases[k]
                ii = np.arange(kk)
                cidx = ch0s[k] + ii // P
                pidx = ii % P
                bval[t, pidx, cidx * P + dpos_s[sl]] = (
                    wdis_s[sl].astype(fp8np))
        # self-diagonal chunk: 2*dis at (slot, chunk NCHUNK-1, col=slot)
        mine = np.where(core_of_node == c)[0]
        tsel = tile_of_node[mine]
        psel = pos_in_bin[mine]
        bval[tsel, psel, (NCHUNK - 1) * P + psel] = (
            (2.0 * dis[mine]).astype(fp8np))

        # gidx blocks per group: [grp: A(t0..t3), B(t0..t3), H(t0..t3)]
        col0 = 0
        for grp in range(NGRP):
            nt = tiles_in(grp)
            for k in range(3):
                cap = caps[k]
                # pads gather safe index 0 (B columns there are zero); the
                # ucode num_idxs register must equal count(idx >= 0)
                flat = np.zeros(nt * cap, np.int64)
                for ti in range(nt):
                    t = GRP * grp + ti
                    kk = int(cls_n[t, k])
                    flat[ti * cap:ti * cap + kk] = cls_idx[t, k, :kk]
                w = flat.reshape(len(flat) // 16, 16).T.astype(np.int16)
                gidx[:, col0:col0 + len(flat) // 16] = np.tile(w, (8, 1))
                col0 += nt * tcols[k]
        assert col0 == GIDX_COLS

        # x slice, transposed, padded, then per-tile blocks [t, p, k*128+j]
        xt = np.zeros((FIN, NLOC), ml_dtypes.bfloat16)
        lslot = tile_of_node[mine] * P + pos_in_bin[mine]
        xt[:, lslot] = x[mine].T.astype(ml_dtypes.bfloat16)
        xtr = np.zeros((NXG * XG, P, 4 * D), ml_dtypes.bfloat16)
        xtr[:TILES] = np.ascontiguousarray(
            xt.reshape(4, P, TILES, P).transpose(2, 1, 0, 3)
        ).reshape(TILES, P, 4 * D)
        xtr = np.ascontiguousarray(
            xtr.reshape(NXG, XG, P, 4 * D).transpose(0, 2, 1, 3)
        ).reshape(NXG, P, XG * 4 * D)
        # per-slot dis (for table scaling)
        dslot = np.zeros((P, TILES), np.float32)
        dslot[lslot % P, lslot // P] = dis[mine]
        in_maps.append({
            "xt": xtr,
            "w1": np.asarray(W1).astype(ml_dtypes.bfloat16),
            "w2": np.asarray(W2).astype(ml_dtypes.bfloat16),
            "wl": np.asarray(Wlin).astype(ml_dtypes.bfloat16),
            "b1": np.asarray(b1, np.float32).reshape(P, 1),
            "b2": np.asarray(b2, np.float32).reshape(P, 1),
            "bl": np.tile(np.asarray(blin, np.float32).reshape(1, NCLS), (P, 1)),
            "gidx": gidx,
            "bval": bval,
            "dslot": dslot,
        })
    return in_maps, core_of_node, tile_of_node, pos_in_bin


def kernel(x, edge_index, W1, b1, W2, b2, Wlin, blin):
    global _PROGRAM, LAST_EXEC_NS
    in_maps, core_of, tile_of, pos_of = _preprocess(
        x, edge_index, W1, b1, W2, b2, Wlin, blin)
    if _PROGRAM is None:
        _PROGRAM = _build_program()
    res = run_bass_kernel_spmd(
        _PROGRAM, in_maps, core_ids=list(range(NCORES)), trace=TRACE)
    LAST_EXEC_NS = res.exec_time_ns
    out = np.empty((N, NCLS), np.float32)
    per_core = [res.results[c]["probs"] for c in range(NCORES)]
    lslot = tile_of * P + pos_of
    for c in range(NCORES):
        mine = np.where(core_of == c)[0]
        out[mine] = per_core[c][lslot[mine]]
    return out


# revision 18
# speedup vs baseline: 1.1297x; 1.0084x over previous
"""2-layer GCN (improved=True) + linear head + softmax on 8 Trainium2 cores.

Strategy (dest-node partitioning, v4: class-phased gather pipeline,
4-tile gather groups):
- Nodes assigned to 8 cores x 49 tiles x 128 slots via balanced bin-packing.
  Self-loops are never gathered: every XW output tile stays resident in SBUF
  (t_all) and the self term is one extra matmul against a per-tile diagonal
  (2*dis_dst) kept in the B matrix.
- Normalization fully folded into data: table rows are dis_src * XW (scaled
  at the PSUM->SBUF copy), the one-hot scatter matrix B holds fp8(dis_dst),
  so aggregation needs no post-scaling at all.
- Table is region-major: region 0 = tiles 0..15 (rows 0:16384), region 1 =
  tiles 16..31 (16384:32768), region 2 = tiles 32..48 (32768:50176).  Each
  layer runs THREE AllGathers (one per region).  Triggers are interleaved
  into the gpsimd gather-issue order mid-stream so they dispatch promptly
  (the gpsimd engine is in-order) while their semaphore deps keep them
  exact.
- Per dest tile, edges go to 3 gather classes with fixed chunk budgets:
  lo-a (5 chunks, sources < 16384, dep AG r0), lo-b (5 chunks, sources <
  32768, dep AG r1), hi (7 chunks, sources >= 17408, dep AG r2).  Gathers
  cover FOUR tiles per instruction (amortizing per-instruction overhead)
  on a strict 4-queue SWDGE rotation, issued class-major.
- Compute is phased per layer: all lo-a chunk matmul groups first
  (recycling gather buffers while AG r1/r2 are in flight), then all lo-b
  groups, then per-tile hi+self groups plus the nonlinearity and the
  next-layer XW.  Each class group drains PSUM into an SBUF f32
  accumulator (acc_all), so only a few PSUM banks are ever live.
- B matrices: even tiles resident in SBUF; odd tiles streamed per class
  with a small lookahead on the scalar queue.
- Head: logits = H2 @ Wlin + blin, softmax over 8 classes.

kernel() is self-contained: host-side numpy does all graph preprocessing;
the device program is identical on all 8 cores, only data differs.
"""
import sys

sys.path.insert(0, "/opt/trn_rl_repo")

import numpy as np
import ml_dtypes

import concourse.bass as bass
import concourse.bacc as bacc
import concourse.mybir as mybir
import concourse.tile as tile
from concourse.tile_rust import add_dep_helper
from concourse.bass_utils import run_bass_kernel_spmd
from concourse.library_config import mlp

# problem constants
N = 50000
E = 800000
FIN = 512
D = 128
NCLS = 8
NCORES = 8

# sharding constants
P = 128
TILES = 49
NLOC = TILES * P            # 6272 slots per core
VTOT = NCORES * NLOC        # 50176 table rows

GRP = 2                     # tiles per gather group
NGRP = (TILES + GRP - 1) // GRP     # 13 (last group has 1 tile)

# region-major table: region r holds tiles RT[r][0]..RT[r][1] of every core
RT = [(0, 16), (16, 32), (32, TILES)]
RROWS = [(b - a) * P for a, b in RT]           # rows per core per region
RBASE = [0, 16384, 32768]                      # global row base per region
HI_BASE = 17408                                # hi gathers read table[HI_BASE:]

# gather classes: chunks per tile
CH_A = 5                    # sources < 16384            (needs AG r0)
CH_B = 5                    # sources < 32768            (needs AG r1)
CH_H = 7                    # sources >= HI_BASE         (needs AG r2)
CAP_A = CH_A * P            # 640
CAP_B = CH_B * P
CAP_H = CH_H * P            # 896
CPT = CH_A + CH_B + CH_H    # 17 gathered chunks per tile
NCHUNK = CPT + 1            # + self-diagonal chunk in B
EDGE_CAP = CPT * P          # 2176 non-self edges per tile

# per-group gidx columns (idx wrapped 16-wide); per tile: 40/40/56 cols
TCOL_A = CAP_A // 16        # 40
TCOL_B = CAP_B // 16
TCOL_H = CAP_H // 16        # 56
TCOLS = TCOL_A + TCOL_B + TCOL_H               # 136 per tile
GIDX_COLS = TILES * TCOLS                      # 6664

# pool depths (groups)
GBUF_A = 8
GBUF_B = 8
GBUF_H = 8
BSL = 8                     # streamed odd-tile B slices per class pool
SLICE_AHEAD = 6             # groups

XG = 4                      # xt tiles per DMA group
NXG = (TILES + XG - 1) // XG

TRACE = False
LAST_EXEC_NS = None

_PROGRAM = None


def tiles_in(grp):
    return min(GRP, TILES - GRP * grp)


def _build_program():
    nc = bacc.Bacc(None, target_bir_lowering=False, num_swdge_queues=4)
    f32 = mybir.dt.float32
    bf16 = mybir.dt.bfloat16
    fp8 = mybir.dt.float8e4

    xt_d = nc.dram_tensor("xt", [NXG, P, XG * 4 * D], bf16,
                          kind="ExternalInput")
    w1_d = nc.dram_tensor("w1", [FIN, D], bf16, kind="ExternalInput")
    w2_d = nc.dram_tensor("w2", [D, D], bf16, kind="ExternalInput")
    wl_d = nc.dram_tensor("wl", [D, NCLS], bf16, kind="ExternalInput")
    b1_d = nc.dram_tensor("b1", [P, 1], f32, kind="ExternalInput")
    b2_d = nc.dram_tensor("b2", [P, 1], f32, kind="ExternalInput")
    bl_d = nc.dram_tensor("bl", [P, NCLS], f32, kind="ExternalInput")
    gidx_d = nc.dram_tensor("gidx", [P, GIDX_COLS], mybir.dt.int16,
                            kind="ExternalInput")
    bval_d = nc.dram_tensor("bval", [TILES, P, NCHUNK * P], fp8,
                            kind="ExternalInput")
    dslot_d = nc.dram_tensor("dslot", [P, TILES], f32, kind="ExternalInput")
    out_d = nc.dram_tensor("probs", [NLOC, NCLS], f32, kind="ExternalOutput")

    with tile.TileContext(nc) as tc:
        lib = nc.gpsimd.load_library(mlp)
        first_gather = [True]
        qctr = [0]
        ni_regs = {n: nc.gpsimd.to_reg(n)
                   for n in (CAP_A, GRP * CAP_A, CAP_H, GRP * CAP_H)}

        from contextlib import ExitStack
        with ExitStack() as stack:
            ep = stack.enter_context
            cp = ep(tc.tile_pool(name="const", bufs=1))
            bpr = ep(tc.tile_pool(name="bres", bufs=1))
            bpa = ep(tc.tile_pool(name="bsla", bufs=BSL))
            bpb = ep(tc.tile_pool(name="bslb", bufs=BSL))
            bph = ep(tc.tile_pool(name="bslh", bufs=BSL))
            glap = ep(tc.tile_pool(name="gla", bufs=GBUF_A))
            glbp = ep(tc.tile_pool(name="glb", bufs=GBUF_B))
            ghip = ep(tc.tile_pool(name="ghi", bufs=GBUF_H))
            up = ep(tc.tile_pool(name="upool", bufs=6))
            hp = ep(tc.tile_pool(name="hpool", bufs=8))
            hdp = ep(tc.tile_pool(name="headp", bufs=3))
            xwps = ep(tc.tile_pool(name="xwps", bufs=2, space="PSUM"))
            aggps = ep(tc.tile_pool(name="aggps", bufs=6, space="PSUM"))
            dr1 = ep(tc.tile_pool(name="dram1", bufs=1, space="DRAM"))
            dr2 = ep(tc.tile_pool(name="dram2", bufs=1, space="DRAM"))
            dr3 = ep(tc.tile_pool(name="dram3", bufs=1, space="DRAM"))
            dr4 = ep(tc.tile_pool(name="dram4", bufs=1, space="DRAM"))
            # ---- constants to SBUF ----
            w1_sb = cp.tile([P, 4 * D], bf16)
            for k in range(4):
                nc.sync.dma_start(w1_sb[:, k * D:(k + 1) * D],
                                  w1_d[k * P:(k + 1) * P, :])
            w2_sb = cp.tile([P, D], bf16)
            nc.sync.dma_start(w2_sb[:], w2_d[:])
            wl_sb = cp.tile([P, NCLS], bf16)
            nc.sync.dma_start(wl_sb[:], wl_d[:])
            b1_sb = cp.tile([P, 1], f32)
            nc.sync.dma_start(b1_sb[:], b1_d[:])
            b2_sb = cp.tile([P, 1], f32)
            nc.sync.dma_start(b2_sb[:], b2_d[:])
            bl_sb = cp.tile([P, NCLS], f32)
            nc.sync.dma_start(bl_sb[:], bl_d[:])
            dslot_sb = cp.tile([P, TILES], f32)
            nc.sync.dma_start(dslot_sb[:], dslot_d[:])
            gidx_sb = cp.tile([P, GIDX_COLS], mybir.dt.int16)
            nc.scalar.dma_start(gidx_sb[:], gidx_d[:])

            # resident XW outputs (layer li overwrites in place per tile)
            t_all = cp.tile([P, TILES * D], bf16)
            # SBUF f32 accumulator per dest tile (overwritten per layer)
            acc_all = cp.tile([P, TILES * P], f32)

            t_loc = [dr1.tile([NLOC, D], bf16, name="t_loc0"),
                     dr2.tile([NLOC, D], bf16, name="t_loc1")]
            t_full = [dr3.tile([VTOT, D], bf16, name="t_full0"),
                      dr4.tile([VTOT, D], bf16, name="t_full1")]
            b_res = {t: bpr.tile([P, NCHUNK * P], fp8, name=f"bres{t}")
                     for t in range(0, TILES, 2)}
            for t in range(0, TILES, 2):
                nc.scalar.dma_start(b_res[t][:], bval_d[t, :, :])

            def cc_ag(li, r):
                a, b = RT[r]
                nc.gpsimd.collective_compute(
                    "AllGather", mybir.AluOpType.bypass,
                    replica_groups=[list(range(NCORES))],
                    ins=[t_loc[li][a * P:b * P, :].opt()],
                    outs=[t_full[li][RBASE[r]:RBASE[r] + NCORES * RROWS[r],
                                     :].opt()],
                )

            # ---- phase 0: XW1 (table rows scaled by dis[src]) ----
            with tc.tile_pool(name="xtp", bufs=2) as xtp:
                for g in range(NXG):
                    xtt = xtp.tile([P, XG * 4 * D], bf16, tag="xt")
                    nc.sync.dma_start(xtt[:], xt_d[g, :, :])
                    for ti in range(XG):
                        t = g * XG + ti
                        if t >= TILES:
                            break
                        ps = xwps.tile([P, D], f32, tag="xw")
                        for k in range(4):
                            nc.tensor.matmul(
                                out=ps[:],
                                lhsT=xtt[:, (ti * 4 + k) * D:(ti * 4 + k + 1) * D],
                                rhs=w1_sb[:, k * D:(k + 1) * D],
                                start=(k == 0), stop=(k == 3),
                            )
                        nc.scalar.activation(
                            out=t_all[:, t * D:(t + 1) * D], in_=ps[:],
                            func=mybir.ActivationFunctionType.Copy,
                            scale=dslot_sb[:, t:t + 1])
                        nc.sync.dma_start(t_loc[0][t * P:(t + 1) * P, :],
                                          t_all[:, t * D:(t + 1) * D])
                        if t == RT[0][1] - 1:
                            cc_ag(0, 0)
                        elif t == RT[1][1] - 1:
                            cc_ag(0, 1)
                        elif t == RT[2][1] - 1:
                            cc_ag(0, 2)

            # ---- gather + B-slice machinery ----
            CLS = {
                "a": dict(pool=glap, bpool=bpa, cap=CAP_A, ch=CH_A,
                          src0=0, src1=16384, tcoff=0, ch0=0, bch=CH_A),
                "b": dict(pool=glbp, bpool=bpb, cap=CAP_B, ch=CH_B,
                          src0=0, src1=32768, tcoff=TCOL_A, ch0=CH_A,
                          bch=CH_B),
                "h": dict(pool=ghip, bpool=bph, cap=CAP_H, ch=CH_H,
                          src0=HI_BASE, src1=VTOT, tcoff=TCOL_A + TCOL_B,
                          ch0=CH_A + CH_B, bch=CH_H + 1),
            }
            # gidx layout: per group g: per class: tiles of the group
            # contiguous.  Column base for (g, class) precomputed:
            GCOL = {}
            col = 0
            for g_ in range(NGRP):
                nt_ = tiles_in(g_)
                for cn_, w_ in (("a", TCOL_A), ("b", TCOL_B), ("h", TCOL_H)):
                    GCOL[(g_, cn_)] = col
                    col += nt_ * w_
            assert col == GIDX_COLS

            g_buf = {}
            bsl_buf = {}

            def issue_gather(li, grp, cname):
                c = CLS[cname]
                nt = tiles_in(grp)
                ni = nt * c["cap"]
                g = c["pool"].tile([P, GRP * c["ch"] * D], bf16, tag=cname,
                                   name=f"g{cname}{li}_{grp}")
                col0 = GCOL[(grp, cname)]
                src = t_full[li][c["src0"]:c["src1"], :]
                qn = qctr[0] % 4
                qctr[0] += 1
                gi = nc.gpsimd.dma_gather(
                    g[:, :ni // P * D].rearrange("p (c d) -> p c d", d=D),
                    src,
                    gidx_sb[:, col0:col0 + ni // 16],
                    ni, ni_regs[ni], D, single_packet=False,
                    queue_num=qn,
                )
                if first_gather[0]:
                    add_dep_helper(gi.ins, lib.ins, reason="lib before gather")
                    first_gather[0] = False
                g_buf[(li, grp, cname)] = g

            # B slices for odd tiles: JIT stream on the scalar queue.
            slice_seq = [(li, cname, grp)
                         for li in (0, 1)
                         for cname in ("a", "b", "h")
                         for grp in range(NGRP)]
            slice_ptr = [0]

            def pump_slices(upto):
                while slice_ptr[0] < min(upto, len(slice_seq)):
                    li, cname, grp = slice_seq[slice_ptr[0]]
                    slice_ptr[0] += 1
                    c = CLS[cname]
                    tiles = []
                    for ti in range(tiles_in(grp)):
                        t = GRP * grp + ti
                        if t % 2 == 0:
                            tiles.append((b_res[t], c["ch0"]))
                        else:
                            bt = c["bpool"].tile([P, c["bch"] * P], fp8,
                                                 tag=cname)
                            nc.scalar.dma_start(
                                bt[:],
                                bval_d[t, :, c["ch0"] * P:
                                       (c["ch0"] + c["bch"]) * P])
                            tiles.append((bt, 0))
                    bsl_buf[(li, cname, grp)] = tiles

            def phase_ab(li, grp, cname):
                """class-chunk matmul group -> acc (copy for a, += for b)."""
                c = CLS[cname]
                g = g_buf.pop((li, grp, cname))
                btl = bsl_buf.pop((li, cname, grp))
                nch = c["ch"]
                for ti in range(tiles_in(grp)):
                    t = GRP * grp + ti
                    bt, bc0 = btl[ti]
                    ps = aggps.tile([P, P], f32, tag="agg")
                    for k in range(nch):
                        nc.tensor.matmul(
                            out=ps[:],
                            lhsT=g[:, (ti * nch + k) * D:
                                   (ti * nch + k + 1) * D],
                            rhs=bt[:, (bc0 + k) * P:(bc0 + k + 1) * P],
                            start=(k == 0), stop=(k == nch - 1),
                        )
                    acc = acc_all[:, t * P:(t + 1) * P]
                    if cname == "a":
                        nc.scalar.activation(
                            out=acc, in_=ps[:],
                            func=mybir.ActivationFunctionType.Copy)
                    else:
                        nc.vector.tensor_add(out=acc, in0=acc, in1=ps[:])

            # phase_h is software-pipelined: the XW2 / head matmul of tile t
            # depends (via relu on scalar) on tile t's agg group, so it is
            # deferred H_LAG tiles so the tensor queue never stalls on the
            # cross-engine chain.
            H_LAG = 5
            h_pend = []

            def emit_tail(li, t, h):
                if li == 0:
                    ps2 = xwps.tile([P, D], f32, tag="xw")
                    nc.tensor.matmul(out=ps2[:], lhsT=h[:], rhs=w2_sb[:],
                                     start=True, stop=True)
                    nc.scalar.activation(
                        out=t_all[:, t * D:(t + 1) * D], in_=ps2[:],
                        func=mybir.ActivationFunctionType.Copy,
                        scale=dslot_sb[:, t:t + 1])
                    nc.sync.dma_start(t_loc[1][t * P:(t + 1) * P, :],
                                      t_all[:, t * D:(t + 1) * D])
                    if t == RT[0][1] - 1:
                        cc_ag(1, 0)
                        for g2 in range(12, 20):
                            issue_gather(0, g2, "h")
                    elif t == RT[1][1] - 1:
                        cc_ag(1, 1)
                        for g2 in range(20, NGRP):
                            issue_gather(0, g2, "h")
                    elif t == RT[2][1] - 1:
                        cc_ag(1, 2)
                        for cn2 in ("a", "b", "h"):
                            for g2 in range(NGRP):
                                issue_gather(1, g2, cn2)
                else:
                    lg = xwps.tile([P, NCLS], f32, tag="xw")
                    nc.tensor.matmul(out=lg[:], lhsT=h[:], rhs=wl_sb[:],
                                     start=True, stop=True)
                    l_sb = hdp.tile([P, NCLS], f32, tag="l")
                    nc.vector.tensor_add(out=l_sb[:], in0=lg[:],
                                         in1=bl_sb[:])
                    nmx = hdp.tile([P, 1], f32, tag="nmx")
                    nc.vector.reduce_max(out=nmx[:], in_=l_sb[:],
                                         axis=mybir.AxisListType.X,
                                         negate=True)
                    e_sb = hdp.tile([P, NCLS], f32, tag="e")
                    nc.scalar.activation(
                        out=e_sb[:], in_=l_sb[:],
                        func=mybir.ActivationFunctionType.Exp,
                        bias=nmx[:])
                    sm = hdp.tile([P, 1], f32, tag="sm")
                    nc.vector.reduce_sum(out=sm[:], in_=e_sb[:],
                                         axis=mybir.AxisListType.X)
                    rs = hdp.tile([P, 1], f32, tag="rs")
                    nc.vector.reciprocal(out=rs[:], in_=sm[:])
                    pr = hdp.tile([P, NCLS], f32, tag="pr")
                    nc.scalar.activation(
                        out=pr[:], in_=e_sb[:],
                        func=mybir.ActivationFunctionType.Copy,
                        scale=rs[:])
                    nc.sync.dma_start(out_d[t * P:(t + 1) * P, :],
                                      pr[:])

            def h_drain(keep):
                while len(h_pend) > keep:
                    li0, t0, h0 = h_pend.pop(0)
                    emit_tail(li0, t0, h0)

            def phase_h(li, grp):
                """hi+self matmul group + relu; tail emitted H_LAG later."""
                g = g_buf.pop((li, grp, "h"))
                btl = bsl_buf.pop((li, "h", grp))
                for ti in range(tiles_in(grp)):
                    t = GRP * grp + ti
                    bt, bc0 = btl[ti]
                    ps = aggps.tile([P, P], f32, tag="agg")
                    for k in range(CH_H):
                        nc.tensor.matmul(
                            out=ps[:],
                            lhsT=g[:, (ti * CH_H + k) * D:
                                   (ti * CH_H + k + 1) * D],
                            rhs=bt[:, (bc0 + k) * P:(bc0 + k + 1) * P],
                            start=(k == 0), stop=False,
                        )
                    # self-diagonal chunk (2*dis_dst) against resident XW
                    nc.tensor.matmul(
                        out=ps[:],
                        lhsT=t_all[:, t * D:(t + 1) * D],
                        rhs=bt[:, (bc0 + CH_H) * P:(bc0 + CH_H + 1) * P],
                        start=False, stop=True,
                    )
                    u = up.tile([P, P], f32, tag="u")
                    nc.vector.tensor_add(out=u[:],
                                         in0=acc_all[:, t * P:(t + 1) * P],
                                         in1=ps[:])
                    bias_sb = b1_sb if li == 0 else b2_sb
                    h = hp.tile([P, P], bf16, tag="h")
                    nc.scalar.activation(out=h[:], in_=u[:],
                                         func=mybir.ActivationFunctionType.Relu,
                                         bias=bias_sb[:])
                    h_pend.append((li, t, h))
                    h_drain(H_LAG)

            # ---- emit both layers ----
            # gpsimd issue order (in-order dispatch!):
            #   L1: a x13, b x13, h g0..5, AGtrig(1,0) [emitted inside
            #   phase_h(g3) at t==15], h g6..9, AG(1,1) [phase_h(g7)],
            #   h g10..12, AG(1,2) [phase_h(g12)], then L2 a, b, h.
            # The triggers are emitted by the phase loop; the gather issues
            # are deferred accordingly so the triggers land between them.
            for cname in ("a", "b"):
                for grp in range(NGRP):
                    issue_gather(0, grp, cname)
            for grp in range(12):
                issue_gather(0, grp, "h")
            pump_slices(SLICE_AHEAD)

            G = [0]

            def step():
                G[0] += 1
                pump_slices(G[0] + SLICE_AHEAD)

            for li in (0, 1):
                for cname in ("a", "b"):
                    for grp in range(NGRP):
                        phase_ab(li, grp, cname)
                        step()
                for grp in range(NGRP):
                    phase_h(li, grp)
                    step()
                h_drain(0)

    nc.compile()
    return nc


def _preprocess(x, edge_index, W1, b1, W2, b2, Wlin, blin):
    """Host-side graph preprocessing -> per-core input dicts + slot maps."""
    fp8np = mybir.dt.np(mybir.dt.float8e4)
    x = np.asarray(x, np.float32)
    ei = np.asarray(edge_index)
    row = ei[0].astype(np.int64)
    col = ei[1].astype(np.int64)

    deg = np.bincount(col, minlength=N).astype(np.float32) + 2.0
    dis = (1.0 / np.sqrt(deg)).astype(np.float32)

    indeg = np.bincount(col, minlength=N)  # non-self in-edges

    # balanced node->bin assignment (bins = core*TILES + tile), snake by degree
    NB = NCORES * TILES
    order = np.argsort(-indeg, kind="stable")
    bin_of_node = np.empty(N, np.int64)
    pos_in_bin = np.empty(N, np.int64)
    full_rounds = N // NB
    rem = N - full_rounds * NB
    fwd = np.arange(NB)
    bwd = fwd[::-1]
    seq = []
    for r in range(full_rounds):
        seq.append(fwd if r % 2 == 0 else bwd)
    if rem:
        seq.append((fwd if full_rounds % 2 == 0 else bwd)[:rem])
    seq = np.concatenate(seq)
    bin_of_node[order] = seq
    srt = np.argsort(bin_of_node, kind="stable")
    cnt = np.bincount(bin_of_node, minlength=NB)
    assert cnt.max() <= P
    starts = np.zeros(NB + 1, np.int64)
    np.cumsum(cnt, out=starts[1:])
    pos_in_bin[srt] = np.arange(N) - starts[bin_of_node[srt]]

    bin_edge_cnt = np.bincount(bin_of_node[col], minlength=NB)
    assert bin_edge_cnt.max() <= EDGE_CAP, (
        f"bin edge overflow: {bin_edge_cnt.max()} > {EDGE_CAP}")

    core_of_node = bin_of_node // TILES
    tile_of_node = bin_of_node % TILES
    # region-major table row
    region_of_tile = np.where(tile_of_node < RT[0][1], 0,
                              np.where(tile_of_node < RT[1][1], 1, 2))
    rbase = np.array(RBASE, np.int64)[region_of_tile]
    rrows = np.array(RROWS, np.int64)[region_of_tile]
    rt0 = np.array([RT[0][0], RT[1][0], RT[2][0]], np.int64)[region_of_tile]
    gslot = (rbase + core_of_node * rrows
             + (tile_of_node - rt0) * P + pos_in_bin)

    # per-edge: destination bin + dest position + dest dis; source table slot
    e_bin = bin_of_node[col]
    e_dpos = pos_in_bin[col]
    e_src = gslot[row]
    e_dis = dis[col]

    e_order = np.argsort(e_bin, kind="stable")
    eb = e_bin[e_order]
    ed = e_dpos[e_order]
    es = e_src[e_order]
    ew = e_dis[e_order]
    bstarts = np.searchsorted(eb, np.arange(NB + 1))

    in_maps = []
    caps = (CAP_A, CAP_B, CAP_H)
    bases = (0, 0, HI_BASE)
    ch0s = (0, CH_A, CH_A + CH_B)
    tcols = (TCOL_A, TCOL_B, TCOL_H)
    for c in range(NCORES):
        gidx = np.zeros((P, GIDX_COLS), np.int16)
        bval = np.zeros((TILES, P, NCHUNK * P), fp8np)
        cls_idx = np.zeros((TILES, 3, CAP_H), np.int64)
        cls_n = np.zeros((TILES, 3), np.int64)
        for t in range(TILES):
            b = c * TILES + t
            lo_f, hi_f = bstarts[b], bstarts[b + 1]
            o = np.argsort(es[lo_f:hi_f], kind="stable")
            srcs = es[lo_f:hi_f][o]
            dpos_s = ed[lo_f:hi_f][o]
            wdis_s = ew[lo_f:hi_f][o]
            ne = len(srcs)
            lo_elig = int((srcs < 32768).sum())
            n_a = min(int((srcs < 16384).sum()), CAP_A)
            n_b = min(lo_elig - n_a, CAP_B)
            rest = ne - n_a - n_b
            assert rest <= CAP_H, f"hi overflow t={t} c={c}: {rest}"
            if rest:
                assert srcs[n_a + n_b] >= HI_BASE, (
                    f"hi source below HI_BASE: {srcs[n_a + n_b]}")
            parts = (slice(0, n_a), slice(n_a, n_a + n_b),
                     slice(n_a + n_b, ne))
            for k in range(3):
                sl = parts[k]
                kk = sl.stop - sl.start
                cls_n[t, k] = kk
                cls_idx[t, k, :kk] = srcs[sl] - bases[k]
                ii = np.arange(kk)
                cidx = ch0s[k] + ii // P
                pidx = ii % P
                bval[t, pidx, cidx * P + dpos_s[sl]] = (
                    wdis_s[sl].astype(fp8np))
        # self-diagonal chunk: 2*dis at (slot, chunk NCHUNK-1, col=slot)
        mine = np.where(core_of_node == c)[0]
        tsel = tile_of_node[mine]
        psel = pos_in_bin[mine]
        bval[tsel, psel, (NCHUNK - 1) * P + psel] = (
            (2.0 * dis[mine]).astype(fp8np))

        # gidx blocks per group: [grp: A(t0..t3), B(t0..t3), H(t0..t3)]
        col0 = 0
        for grp in range(NGRP):
            nt = tiles_in(grp)
            for k in range(3):
                cap = caps[k]
                # pads gather safe index 0 (B columns there are zero); the
                # ucode num_idxs register must equal count(idx >= 0)
                flat = np.zeros(nt * cap, np.int64)
                for ti in range(nt):
                    t = GRP * grp + ti
                    kk = int(cls_n[t, k])
                    flat[ti * cap:ti * cap + kk] = cls_idx[t, k, :kk]
                w = flat.reshape(len(flat) // 16, 16).T.astype(np.int16)
                gidx[:, col0:col0 + len(flat) // 16] = np.tile(w, (8, 1))
                col0 += nt * tcols[k]
        assert col0 == GIDX_COLS

        # x slice, transposed, padded, then per-tile blocks [t, p, k*128+j]
        xt = np.zeros((FIN, NLOC), ml_dtypes.bfloat16)
        lslot = tile_of_node[mine] * P + pos_in_bin[mine]
        xt[:, lslot] = x[mine].T.astype(ml_dtypes.bfloat16)
        xtr = np.zeros((NXG * XG, P, 4 * D), ml_dtypes.bfloat16)
        xtr[:TILES] = np.ascontiguousarray(
            xt.reshape(4, P, TILES, P).transpose(2, 1, 0, 3)
        ).reshape(TILES, P, 4 * D)
        xtr = np.ascontiguousarray(
            xtr.reshape(NXG, XG, P, 4 * D).transpose(0, 2, 1, 3)
        ).reshape(NXG, P, XG * 4 * D)
        # per-slot dis (for table scaling)
        dslot = np.zeros((P, TILES), np.float32)
        dslot[lslot % P, lslot // P] = dis[mine]
        in_maps.append({
            "xt": xtr,
            "w1": np.asarray(W1).astype(ml_dtypes.bfloat16),
            "w2": np.asarray(W2).astype(ml_dtypes.bfloat16),
            "wl": np.asarray(Wlin).astype(ml_dtypes.bfloat16),
            "b1": np.asarray(b1, np.float32).reshape(P, 1),
            "b2": np.asarray(b2, np.float32).reshape(P, 1),
            "bl": np.tile(np.asarray(blin, np.float32).reshape(1, NCLS), (P, 1)),
            "gidx": gidx,
            "bval": bval,
            "dslot": dslot,
        })
    return in_maps, core_of_node, tile_of_node, pos_in_bin


def kernel(x, edge_index, W1, b1, W2, b2, Wlin, blin):
    global _PROGRAM, LAST_EXEC_NS
    in_maps, core_of, tile_of, pos_of = _preprocess(
        x, edge_index, W1, b1, W2, b2, Wlin, blin)
    if _PROGRAM is None:
        _PROGRAM = _build_program()
    res = run_bass_kernel_spmd(
        _PROGRAM, in_maps, core_ids=list(range(NCORES)), trace=TRACE)
    LAST_EXEC_NS = res.exec_time_ns
    out = np.empty((N, NCLS), np.float32)
    per_core = [res.results[c]["probs"] for c in range(NCORES)]
    lslot = tile_of * P + pos_of
    for c in range(NCORES):
        mine = np.where(core_of == c)[0]
        out[mine] = per_core[c][lslot[mine]]
    return out


# revision 20
# speedup vs baseline: 1.1427x; 1.0115x over previous
"""2-layer GCN (improved=True) + linear head + softmax on 8 Trainium2 cores.

Strategy (dest-node partitioning, v4: class-phased gather pipeline,
4-tile gather groups):
- Nodes assigned to 8 cores x 49 tiles x 128 slots via balanced bin-packing.
  Self-loops are never gathered: every XW output tile stays resident in SBUF
  (t_all) and the self term is one extra matmul against a per-tile diagonal
  (2*dis_dst) kept in the B matrix.
- Normalization fully folded into data: table rows are dis_src * XW (scaled
  at the PSUM->SBUF copy), the one-hot scatter matrix B holds fp8(dis_dst),
  so aggregation needs no post-scaling at all.
- Table is region-major: region 0 = tiles 0..15 (rows 0:16384), region 1 =
  tiles 16..31 (16384:32768), region 2 = tiles 32..48 (32768:50176).  Each
  layer runs THREE AllGathers (one per region).  Triggers are interleaved
  into the gpsimd gather-issue order mid-stream so they dispatch promptly
  (the gpsimd engine is in-order) while their semaphore deps keep them
  exact.
- Per dest tile, edges go to 3 gather classes with fixed chunk budgets:
  lo-a (5 chunks, sources < 16384, dep AG r0), lo-b (5 chunks, sources <
  32768, dep AG r1), hi (7 chunks, sources >= 17408, dep AG r2).  Gathers
  cover FOUR tiles per instruction (amortizing per-instruction overhead)
  on a strict 4-queue SWDGE rotation, issued class-major.
- Compute is phased per layer: all lo-a chunk matmul groups first
  (recycling gather buffers while AG r1/r2 are in flight), then all lo-b
  groups, then per-tile hi+self groups plus the nonlinearity and the
  next-layer XW.  Each class group drains PSUM into an SBUF f32
  accumulator (acc_all), so only a few PSUM banks are ever live.
- B matrices: even tiles resident in SBUF; odd tiles streamed per class
  with a small lookahead on the scalar queue.
- Head: logits = H2 @ Wlin + blin, softmax over 8 classes.

kernel() is self-contained: host-side numpy does all graph preprocessing;
the device program is identical on all 8 cores, only data differs.
"""
import sys

sys.path.insert(0, "/opt/trn_rl_repo")

import numpy as np
import ml_dtypes

import concourse.bass as bass
import concourse.bacc as bacc
import concourse.mybir as mybir
import concourse.tile as tile
from concourse.tile_rust import add_dep_helper
from concourse.bass_utils import run_bass_kernel_spmd
from concourse.library_config import mlp

# problem constants
N = 50000
E = 800000
FIN = 512
D = 128
NCLS = 8
NCORES = 8

# sharding constants
P = 128
TILES = 49
NLOC = TILES * P            # 6272 slots per core
VTOT = NCORES * NLOC        # 50176 table rows

GRP = 2                     # tiles per gather group
NGRP = (TILES + GRP - 1) // GRP     # 13 (last group has 1 tile)

# region-major table: region r holds tiles RT[r][0]..RT[r][1] of every core
RT = [(0, 16), (16, 32), (32, TILES)]
RROWS = [(b - a) * P for a, b in RT]           # rows per core per region
RBASE = [0, 16384, 32768]                      # global row base per region
HI_BASE = 17408                                # hi gathers read table[HI_BASE:]

# gather classes: chunks per tile
CH_A = 5                    # sources < 16384            (needs AG r0)
CH_B = 5                    # sources < 32768            (needs AG r1)
CH_H = 7                    # sources >= HI_BASE         (needs AG r2)
CAP_A = CH_A * P            # 640
CAP_B = CH_B * P
CAP_H = CH_H * P            # 896
CPT = CH_A + CH_B + CH_H    # 17 gathered chunks per tile
NCHUNK = CPT + 1            # + self-diagonal chunk in B
EDGE_CAP = CPT * P          # 2176 non-self edges per tile

# per-group gidx columns (idx wrapped 16-wide); per tile: 40/40/56 cols
TCOL_A = CAP_A // 16        # 40
TCOL_B = CAP_B // 16
TCOL_H = CAP_H // 16        # 56
TCOLS = TCOL_A + TCOL_B + TCOL_H               # 136 per tile
GIDX_COLS = TILES * TCOLS                      # 6664

# pool depths (groups)
GBUF_A = 8
GBUF_B = 8
GBUF_H = 8
BSL = 8                     # streamed odd-tile B slices per class pool
SLICE_AHEAD = 6             # groups

XG = 4                      # xt tiles per DMA group
NXG = (TILES + XG - 1) // XG

TRACE = False
LAST_EXEC_NS = None

_PROGRAM = None


def tiles_in(grp):
    return min(GRP, TILES - GRP * grp)


def _build_program():
    nc = bacc.Bacc(None, target_bir_lowering=False, num_swdge_queues=4)
    f32 = mybir.dt.float32
    bf16 = mybir.dt.bfloat16
    fp8 = mybir.dt.float8e4

    xt_d = nc.dram_tensor("xt", [NXG, P, XG * 4 * D], bf16,
                          kind="ExternalInput")
    w1_d = nc.dram_tensor("w1", [FIN, D], bf16, kind="ExternalInput")
    w2_d = nc.dram_tensor("w2", [D, D], bf16, kind="ExternalInput")
    wl_d = nc.dram_tensor("wl", [D, NCLS], bf16, kind="ExternalInput")
    b1_d = nc.dram_tensor("b1", [P, 1], f32, kind="ExternalInput")
    b2_d = nc.dram_tensor("b2", [P, 1], f32, kind="ExternalInput")
    bl_d = nc.dram_tensor("bl", [P, NCLS], f32, kind="ExternalInput")
    gidx_d = nc.dram_tensor("gidx", [P, GIDX_COLS], mybir.dt.int16,
                            kind="ExternalInput")
    bval_d = nc.dram_tensor("bval", [TILES, P, NCHUNK * P], fp8,
                            kind="ExternalInput")
    dslot_d = nc.dram_tensor("dslot", [P, TILES], f32, kind="ExternalInput")
    out_d = nc.dram_tensor("probs", [NLOC, NCLS], f32, kind="ExternalOutput")

    with tile.TileContext(nc) as tc:
        lib = nc.gpsimd.load_library(mlp)
        first_gather = [True]
        qctr = [0]
        ni_regs = {n: nc.gpsimd.to_reg(n)
                   for n in (CAP_A, GRP * CAP_A, CAP_H, GRP * CAP_H)}

        from contextlib import ExitStack
        with ExitStack() as stack:
            ep = stack.enter_context
            cp = ep(tc.tile_pool(name="const", bufs=1))
            bpr = ep(tc.tile_pool(name="bres", bufs=1))
            bpa = ep(tc.tile_pool(name="bsla", bufs=BSL))
            bpb = ep(tc.tile_pool(name="bslb", bufs=BSL))
            bph = ep(tc.tile_pool(name="bslh", bufs=BSL))
            glap = ep(tc.tile_pool(name="gla", bufs=GBUF_A))
            glbp = ep(tc.tile_pool(name="glb", bufs=GBUF_B))
            ghip = ep(tc.tile_pool(name="ghi", bufs=GBUF_H))
            up = ep(tc.tile_pool(name="upool", bufs=6))
            hp = ep(tc.tile_pool(name="hpool", bufs=8))
            hdp = ep(tc.tile_pool(name="headp", bufs=3))
            xwps = ep(tc.tile_pool(name="xwps", bufs=2, space="PSUM"))
            aggps = ep(tc.tile_pool(name="aggps", bufs=6, space="PSUM"))
            dr1 = ep(tc.tile_pool(name="dram1", bufs=1, space="DRAM"))
            dr2 = ep(tc.tile_pool(name="dram2", bufs=1, space="DRAM"))
            dr3 = ep(tc.tile_pool(name="dram3", bufs=1, space="DRAM"))
            dr4 = ep(tc.tile_pool(name="dram4", bufs=1, space="DRAM"))
            # ---- constants to SBUF ----
            w1_sb = cp.tile([P, 4 * D], bf16)
            for k in range(4):
                nc.sync.dma_start(w1_sb[:, k * D:(k + 1) * D],
                                  w1_d[k * P:(k + 1) * P, :])
            w2_sb = cp.tile([P, D], bf16)
            nc.sync.dma_start(w2_sb[:], w2_d[:])
            wl_sb = cp.tile([P, NCLS], bf16)
            nc.sync.dma_start(wl_sb[:], wl_d[:])
            b1_sb = cp.tile([P, 1], f32)
            nc.sync.dma_start(b1_sb[:], b1_d[:])
            b2_sb = cp.tile([P, 1], f32)
            nc.sync.dma_start(b2_sb[:], b2_d[:])
            bl_sb = cp.tile([P, NCLS], f32)
            nc.sync.dma_start(bl_sb[:], bl_d[:])
            dslot_sb = cp.tile([P, TILES], f32)
            nc.sync.dma_start(dslot_sb[:], dslot_d[:])
            gidx_sb = cp.tile([P, GIDX_COLS], mybir.dt.int16)
            nc.scalar.dma_start(gidx_sb[:], gidx_d[:])

            # resident XW outputs (layer li overwrites in place per tile)
            t_all = cp.tile([P, TILES * D], bf16)
            # SBUF f32 accumulator per dest tile (overwritten per layer)
            acc_all = cp.tile([P, TILES * P], f32)

            t_loc = [dr1.tile([NLOC, D], bf16, name="t_loc0"),
                     dr2.tile([NLOC, D], bf16, name="t_loc1")]
            t_full = [dr3.tile([VTOT, D], bf16, name="t_full0"),
                      dr4.tile([VTOT, D], bf16, name="t_full1")]
            b_res = {t: bpr.tile([P, NCHUNK * P], fp8, name=f"bres{t}")
                     for t in range(0, TILES, 2)}
            for t in range(0, TILES, 2):
                nc.scalar.dma_start(b_res[t][:], bval_d[t, :, :])

            def cc_ag(li, r):
                a, b = RT[r]
                nc.gpsimd.collective_compute(
                    "AllGather", mybir.AluOpType.bypass,
                    replica_groups=[list(range(NCORES))],
                    ins=[t_loc[li][a * P:b * P, :].opt()],
                    outs=[t_full[li][RBASE[r]:RBASE[r] + NCORES * RROWS[r],
                                     :].opt()],
                )

            # ---- phase 0: XW1 (table rows scaled by dis[src]) ----
            with tc.tile_pool(name="xtp", bufs=2) as xtp:
                for g in range(NXG):
                    xtt = xtp.tile([P, XG * 4 * D], bf16, tag="xt")
                    nc.sync.dma_start(xtt[:], xt_d[g, :, :])
                    for ti in range(XG):
                        t = g * XG + ti
                        if t >= TILES:
                            break
                        ps = xwps.tile([P, D], f32, tag="xw")
                        for k in range(4):
                            nc.tensor.matmul(
                                out=ps[:],
                                lhsT=xtt[:, (ti * 4 + k) * D:(ti * 4 + k + 1) * D],
                                rhs=w1_sb[:, k * D:(k + 1) * D],
                                start=(k == 0), stop=(k == 3),
                            )
                        nc.scalar.activation(
                            out=t_all[:, t * D:(t + 1) * D], in_=ps[:],
                            func=mybir.ActivationFunctionType.Copy,
                            scale=dslot_sb[:, t:t + 1])
                        nc.sync.dma_start(t_loc[0][t * P:(t + 1) * P, :],
                                          t_all[:, t * D:(t + 1) * D])
                        if t == RT[0][1] - 1:
                            cc_ag(0, 0)
                        elif t == RT[1][1] - 1:
                            cc_ag(0, 1)
                        elif t == RT[2][1] - 1:
                            cc_ag(0, 2)

            # ---- gather + B-slice machinery ----
            CLS = {
                "a": dict(pool=glap, bpool=bpa, cap=CAP_A, ch=CH_A,
                          src0=0, src1=16384, tcoff=0, ch0=0, bch=CH_A),
                "b": dict(pool=glbp, bpool=bpb, cap=CAP_B, ch=CH_B,
                          src0=0, src1=32768, tcoff=TCOL_A, ch0=CH_A,
                          bch=CH_B),
                "h": dict(pool=ghip, bpool=bph, cap=CAP_H, ch=CH_H,
                          src0=HI_BASE, src1=VTOT, tcoff=TCOL_A + TCOL_B,
                          ch0=CH_A + CH_B, bch=CH_H + 1),
            }
            # gidx layout: per group g: per class: tiles of the group
            # contiguous.  Column base for (g, class) precomputed:
            GCOL = {}
            col = 0
            for g_ in range(NGRP):
                nt_ = tiles_in(g_)
                for cn_, w_ in (("a", TCOL_A), ("b", TCOL_B), ("h", TCOL_H)):
                    GCOL[(g_, cn_)] = col
                    col += nt_ * w_
            assert col == GIDX_COLS

            g_buf = {}
            bsl_buf = {}

            def issue_gather(li, grp, cname):
                c = CLS[cname]
                nt = tiles_in(grp)
                ni = nt * c["cap"]
                g = c["pool"].tile([P, GRP * c["ch"] * D], bf16, tag=cname,
                                   name=f"g{cname}{li}_{grp}")
                col0 = GCOL[(grp, cname)]
                src = t_full[li][c["src0"]:c["src1"], :]
                qn = qctr[0] % 4
                qctr[0] += 1
                gi = nc.gpsimd.dma_gather(
                    g[:, :ni // P * D].rearrange("p (c d) -> p c d", d=D),
                    src,
                    gidx_sb[:, col0:col0 + ni // 16],
                    ni, ni_regs[ni], D, single_packet=False,
                    queue_num=qn,
                )
                if first_gather[0]:
                    add_dep_helper(gi.ins, lib.ins, reason="lib before gather")
                    first_gather[0] = False
                g_buf[(li, grp, cname)] = g

            # B slices for odd tiles: JIT stream on the scalar queue.
            slice_seq = [(li, cname, grp)
                         for li in (0, 1)
                         for cname in ("a", "b", "h")
                         for grp in range(NGRP)]
            slice_ptr = [0]

            def pump_slices(upto):
                while slice_ptr[0] < min(upto, len(slice_seq)):
                    li, cname, grp = slice_seq[slice_ptr[0]]
                    slice_ptr[0] += 1
                    c = CLS[cname]
                    tiles = []
                    for ti in range(tiles_in(grp)):
                        t = GRP * grp + ti
                        if t % 2 == 0:
                            tiles.append((b_res[t], c["ch0"]))
                        else:
                            bt = c["bpool"].tile([P, c["bch"] * P], fp8,
                                                 tag=cname)
                            nc.scalar.dma_start(
                                bt[:],
                                bval_d[t, :, c["ch0"] * P:
                                       (c["ch0"] + c["bch"]) * P])
                            tiles.append((bt, 0))
                    bsl_buf[(li, cname, grp)] = tiles

            def phase_ab(li, grp, cname):
                """class-chunk matmul group -> acc (copy for a, += for b)."""
                c = CLS[cname]
                g = g_buf.pop((li, grp, cname))
                btl = bsl_buf.pop((li, cname, grp))
                nch = c["ch"]
                for ti in range(tiles_in(grp)):
                    t = GRP * grp + ti
                    bt, bc0 = btl[ti]
                    ps = aggps.tile([P, P], f32, tag="agg")
                    for k in range(nch):
                        nc.tensor.matmul(
                            out=ps[:],
                            lhsT=g[:, (ti * nch + k) * D:
                                   (ti * nch + k + 1) * D],
                            rhs=bt[:, (bc0 + k) * P:(bc0 + k + 1) * P],
                            start=(k == 0), stop=(k == nch - 1),
                        )
                    acc = acc_all[:, t * P:(t + 1) * P]
                    if cname == "a":
                        nc.scalar.activation(
                            out=acc, in_=ps[:],
                            func=mybir.ActivationFunctionType.Copy)
                    else:
                        nc.vector.tensor_add(out=acc, in0=acc, in1=ps[:])

            # phase_h is software-pipelined: the XW2 / head matmul of tile t
            # depends (via relu on scalar) on tile t's agg group, so it is
            # deferred H_LAG tiles so the tensor queue never stalls on the
            # cross-engine chain.
            H_LAG = 5
            h_pend = []

            def emit_tail(li, t, h):
                if li == 0:
                    ps2 = xwps.tile([P, D], f32, tag="xw")
                    nc.tensor.matmul(out=ps2[:], lhsT=h[:], rhs=w2_sb[:],
                                     start=True, stop=True)
                    nc.scalar.activation(
                        out=t_all[:, t * D:(t + 1) * D], in_=ps2[:],
                        func=mybir.ActivationFunctionType.Copy,
                        scale=dslot_sb[:, t:t + 1])
                    nc.sync.dma_start(t_loc[1][t * P:(t + 1) * P, :],
                                      t_all[:, t * D:(t + 1) * D])
                    if t == RT[0][1] - 1:
                        cc_ag(1, 0)
                        for g2 in range(12, 20):
                            issue_gather(0, g2, "h")
                    elif t == RT[1][1] - 1:
                        cc_ag(1, 1)
                        for g2 in range(20, NGRP):
                            issue_gather(0, g2, "h")
                    elif t == RT[2][1] - 1:
                        cc_ag(1, 2)
                        for cn2 in ("a", "b", "h"):
                            for g2 in range(NGRP):
                                issue_gather(1, g2, cn2)
                else:
                    lg = xwps.tile([P, NCLS], f32, tag="xw")
                    nc.tensor.matmul(out=lg[:], lhsT=h[:], rhs=wl_sb[:],
                                     start=True, stop=True)
                    l_sb = hdp.tile([P, NCLS], f32, tag="l")
                    nc.vector.tensor_add(out=l_sb[:], in0=lg[:],
                                         in1=bl_sb[:])
                    nmx = hdp.tile([P, 1], f32, tag="nmx")
                    nc.vector.reduce_max(out=nmx[:], in_=l_sb[:],
                                         axis=mybir.AxisListType.X,
                                         negate=True)
                    e_sb = hdp.tile([P, NCLS], f32, tag="e")
                    nc.scalar.activation(
                        out=e_sb[:], in_=l_sb[:],
                        func=mybir.ActivationFunctionType.Exp,
                        bias=nmx[:])
                    sm = hdp.tile([P, 1], f32, tag="sm")
                    nc.vector.reduce_sum(out=sm[:], in_=e_sb[:],
                                         axis=mybir.AxisListType.X)
                    rs = hdp.tile([P, 1], f32, tag="rs")
                    nc.vector.reciprocal(out=rs[:], in_=sm[:])
                    pr = hdp.tile([P, NCLS], f32, tag="pr")
                    nc.scalar.activation(
                        out=pr[:], in_=e_sb[:],
                        func=mybir.ActivationFunctionType.Copy,
                        scale=rs[:])
                    nc.sync.dma_start(out_d[t * P:(t + 1) * P, :],
                                      pr[:])

            def h_drain(keep):
                while len(h_pend) > keep:
                    li0, t0, h0 = h_pend.pop(0)
                    emit_tail(li0, t0, h0)

            def phase_h(li, grp):
                """hi+self matmul group + relu; tail emitted H_LAG later."""
                g = g_buf.pop((li, grp, "h"))
                btl = bsl_buf.pop((li, "h", grp))
                for ti in range(tiles_in(grp)):
                    t = GRP * grp + ti
                    bt, bc0 = btl[ti]
                    ps = aggps.tile([P, P], f32, tag="agg")
                    for k in range(CH_H):
                        nc.tensor.matmul(
                            out=ps[:],
                            lhsT=g[:, (ti * CH_H + k) * D:
                                   (ti * CH_H + k + 1) * D],
                            rhs=bt[:, (bc0 + k) * P:(bc0 + k + 1) * P],
                            start=(k == 0), stop=False,
                        )
                    # self-diagonal chunk (2*dis_dst) against resident XW
                    nc.tensor.matmul(
                        out=ps[:],
                        lhsT=t_all[:, t * D:(t + 1) * D],
                        rhs=bt[:, (bc0 + CH_H) * P:(bc0 + CH_H + 1) * P],
                        start=False, stop=True,
                    )
                    u = up.tile([P, P], f32, tag="u")
                    nc.vector.tensor_add(out=u[:],
                                         in0=acc_all[:, t * P:(t + 1) * P],
                                         in1=ps[:])
                    bias_sb = b1_sb if li == 0 else b2_sb
                    h = hp.tile([P, P], bf16, tag="h")
                    nc.scalar.activation(out=h[:], in_=u[:],
                                         func=mybir.ActivationFunctionType.Relu,
                                         bias=bias_sb[:])
                    h_pend.append((li, t, h))
                    h_drain(H_LAG)

            # ---- emit both layers ----
            # gpsimd issue order (in-order dispatch!):
            #   L1: a x13, b x13, h g0..5, AGtrig(1,0) [emitted inside
            #   phase_h(g3) at t==15], h g6..9, AG(1,1) [phase_h(g7)],
            #   h g10..12, AG(1,2) [phase_h(g12)], then L2 a, b, h.
            # The triggers are emitted by the phase loop; the gather issues
            # are deferred accordingly so the triggers land between them.
            for cname in ("a", "b"):
                for grp in range(NGRP):
                    issue_gather(0, grp, cname)
            for grp in range(12):
                issue_gather(0, grp, "h")
            pump_slices(SLICE_AHEAD)

            G = [0]

            def step():
                G[0] += 1
                pump_slices(G[0] + SLICE_AHEAD)

            for li in (0, 1):
                for cname in ("a", "b"):
                    for grp in range(NGRP):
                        phase_ab(li, grp, cname)
                        step()
                for grp in range(NGRP):
                    phase_h(li, grp)
                    step()
                h_drain(0)

    nc.compile()
    return nc


def _preprocess(x, edge_index, W1, b1, W2, b2, Wlin, blin):
    """Host-side graph preprocessing -> per-core input dicts + slot maps."""
    fp8np = mybir.dt.np(mybir.dt.float8e4)
    x = np.asarray(x, np.float32)
    ei = np.asarray(edge_index)
    row = ei[0].astype(np.int64)
    col = ei[1].astype(np.int64)

    deg = np.bincount(col, minlength=N).astype(np.float32) + 2.0
    dis = (1.0 / np.sqrt(deg)).astype(np.float32)

    indeg = np.bincount(col, minlength=N)  # non-self in-edges

    # balanced node->bin assignment (bins = core*TILES + tile), snake by degree
    NB = NCORES * TILES
    order = np.argsort(-indeg, kind="stable")
    bin_of_node = np.empty(N, np.int64)
    pos_in_bin = np.empty(N, np.int64)
    full_rounds = N // NB
    rem = N - full_rounds * NB
    fwd = np.arange(NB)
    bwd = fwd[::-1]
    seq = []
    for r in range(full_rounds):
        seq.append(fwd if r % 2 == 0 else bwd)
    if rem:
        seq.append((fwd if full_rounds % 2 == 0 else bwd)[:rem])
    seq = np.concatenate(seq)
    bin_of_node[order] = seq
    srt = np.argsort(bin_of_node, kind="stable")
    cnt = np.bincount(bin_of_node, minlength=NB)
    assert cnt.max() <= P
    starts = np.zeros(NB + 1, np.int64)
    np.cumsum(cnt, out=starts[1:])
    pos_in_bin[srt] = np.arange(N) - starts[bin_of_node[srt]]

    bin_edge_cnt = np.bincount(bin_of_node[col], minlength=NB)
    assert bin_edge_cnt.max() <= EDGE_CAP, (
        f"bin edge overflow: {bin_edge_cnt.max()} > {EDGE_CAP}")

    core_of_node = bin_of_node // TILES
    tile_of_node = bin_of_node % TILES
    # region-major table row
    region_of_tile = np.where(tile_of_node < RT[0][1], 0,
                              np.where(tile_of_node < RT[1][1], 1, 2))
    rbase = np.array(RBASE, np.int64)[region_of_tile]
    rrows = np.array(RROWS, np.int64)[region_of_tile]
    rt0 = np.array([RT[0][0], RT[1][0], RT[2][0]], np.int64)[region_of_tile]
    gslot = (rbase + core_of_node * rrows
             + (tile_of_node - rt0) * P + pos_in_bin)

    # per-edge: destination bin + dest position + dest dis; source table slot
    e_bin = bin_of_node[col]
    e_dpos = pos_in_bin[col]
    e_src = gslot[row]
    e_dis = dis[col]

    e_order = np.argsort(e_bin, kind="stable")
    eb = e_bin[e_order]
    ed = e_dpos[e_order]
    es = e_src[e_order]
    ew = e_dis[e_order]
    bstarts = np.searchsorted(eb, np.arange(NB + 1))

    in_maps = []
    caps = (CAP_A, CAP_B, CAP_H)
    bases = (0, 0, HI_BASE)
    ch0s = (0, CH_A, CH_A + CH_B)
    tcols = (TCOL_A, TCOL_B, TCOL_H)
    for c in range(NCORES):
        gidx = np.zeros((P, GIDX_COLS), np.int16)
        bval = np.zeros((TILES, P, NCHUNK * P), fp8np)
        cls_idx = np.zeros((TILES, 3, CAP_H), np.int64)
        cls_n = np.zeros((TILES, 3), np.int64)
        for t in range(TILES):
            b = c * TILES + t
            lo_f, hi_f = bstarts[b], bstarts[b + 1]
            o = np.argsort(es[lo_f:hi_f], kind="stable")
            srcs = es[lo_f:hi_f][o]
            dpos_s = ed[lo_f:hi_f][o]
            wdis_s = ew[lo_f:hi_f][o]
            ne = len(srcs)
            lo_elig = int((srcs < 32768).sum())
            n_a = min(int((srcs < 16384).sum()), CAP_A)
            n_b = min(lo_elig - n_a, CAP_B)
            rest = ne - n_a - n_b
            assert rest <= CAP_H, f"hi overflow t={t} c={c}: {rest}"
            if rest:
                assert srcs[n_a + n_b] >= HI_BASE, (
                    f"hi source below HI_BASE: {srcs[n_a + n_b]}")
            parts = (slice(0, n_a), slice(n_a, n_a + n_b),
                     slice(n_a + n_b, ne))
            for k in range(3):
                sl = parts[k]
                kk = sl.stop - sl.start
                cls_n[t, k] = kk
                cls_idx[t, k, :kk] = srcs[sl] - bases[k]
                ii = np.arange(kk)
                cidx = ch0s[k] + ii // P
                pidx = ii % P
                bval[t, pidx, cidx * P + dpos_s[sl]] = (
                    wdis_s[sl].astype(fp8np))
        # self-diagonal chunk: 2*dis at (slot, chunk NCHUNK-1, col=slot)
        mine = np.where(core_of_node == c)[0]
        tsel = tile_of_node[mine]
        psel = pos_in_bin[mine]
        bval[tsel, psel, (NCHUNK - 1) * P + psel] = (
            (2.0 * dis[mine]).astype(fp8np))

        # gidx blocks per group: [grp: A(t0..t3), B(t0..t3), H(t0..t3)]
        col0 = 0
        for grp in range(NGRP):
            nt = tiles_in(grp)
            for k in range(3):
                cap = caps[k]
                # pads gather safe index 0 (B columns there are zero); the
                # ucode num_idxs register must equal count(idx >= 0)
                flat = np.zeros(nt * cap, np.int64)
                for ti in range(nt):
                    t = GRP * grp + ti
                    kk = int(cls_n[t, k])
                    flat[ti * cap:ti * cap + kk] = cls_idx[t, k, :kk]
                w = flat.reshape(len(flat) // 16, 16).T.astype(np.int16)
                gidx[:, col0:col0 + len(flat) // 16] = np.tile(w, (8, 1))
                col0 += nt * tcols[k]
        assert col0 == GIDX_COLS

        # x slice, transposed, padded, then per-tile blocks [t, p, k*128+j]
        xt = np.zeros((FIN, NLOC), ml_dtypes.bfloat16)
        lslot = tile_of_node[mine] * P + pos_in_bin[mine]
        xt[:, lslot] = x[mine].T.astype(ml_dtypes.bfloat16)
        xtr = np.zeros((NXG * XG, P, 4 * D), ml_dtypes.bfloat16)
        xtr[:TILES] = np.ascontiguousarray(
            xt.reshape(4, P, TILES, P).transpose(2, 1, 0, 3)
        ).reshape(TILES, P, 4 * D)
        xtr = np.ascontiguousarray(
            xtr.reshape(NXG, XG, P, 4 * D).transpose(0, 2, 1, 3)
        ).reshape(NXG, P, XG * 4 * D)
        # per-slot dis (for table scaling)
        dslot = np.zeros((P, TILES), np.float32)
        dslot[lslot % P, lslot // P] = dis[mine]
        in_maps.append({
            "xt": xtr,
            "w1": np.asarray(W1).astype(ml_dtypes.bfloat16),
            "w2": np.asarray(W2).astype(ml_dtypes.bfloat16),
            "wl": np.asarray(Wlin).astype(ml_dtypes.bfloat16),
            "b1": np.asarray(b1, np.float32).reshape(P, 1),
            "b2": np.asarray(b2, np.float32).reshape(P, 1),
            "bl": np.tile(np.asarray(blin, np.float32).reshape(1, NCLS), (P, 1)),
            "gidx": gidx,
            "bval": bval,
            "dslot": dslot,
        })
    return in_maps, core_of_node, tile_of_node, pos_in_bin


def kernel(x, edge_index, W1, b1, W2, b2, Wlin, blin):
    global _PROGRAM, LAST_EXEC_NS
    in_maps, core_of, tile_of, pos_of = _preprocess(
        x, edge_index, W1, b1, W2, b2, Wlin, blin)
    if _PROGRAM is None:
        _PROGRAM = _build_program()
    res = run_bass_kernel_spmd(
        _PROGRAM, in_maps, core_ids=list(range(NCORES)), trace=TRACE)
    LAST_EXEC_NS = res.exec_time_ns
    out = np.empty((N, NCLS), np.float32)
    per_core = [res.results[c]["probs"] for c in range(NCORES)]
    lslot = tile_of * P + pos_of
    for c in range(NCORES):
        mine = np.where(core_of == c)[0]
        out[mine] = per_core[c][lslot[mine]]
    return out
